# revision 5
# baseline (speedup 1.0000x reference)
"""DeepSeek-V2-Lite matrix-absorbed MLA decode on 8 Trainium2 NeuronCores.

Sharding: data-parallel over batch (4 sequences per core), weights replicated.
Host-side input prep casts the KV caches and weights to bf16 and ships the
compressed-KV cache in both natural [k, c] and transposed [c, k] layouts so
both attention matmuls stream through the PE without any on-device transposes
of large tensors. Attention is a single flash pass: softmax skips the max
subtraction (|scores*scale| <= ~4 for this problem family, exp stays finite
in fp32) and the denominator comes from the Exp activation's accum_out.
"""

import sys

import numpy as np
import ml_dtypes

for _p in ("/opt/trn_rl_repo",):
    if _p not in sys.path:
        sys.path.insert(0, _p)

import concourse.bass as bass  # noqa: E402
import concourse.mybir as mybir  # noqa: E402
import concourse.tile as tile  # noqa: E402
from concourse import bacc  # noqa: E402
from concourse.bass_utils import run_bass_kernel_spmd  # noqa: E402
from concourse.masks import make_identity  # noqa: E402

# Problem constants (hardcoded per harness contract).
H = 2048
NH = 16
DR = 64
DC = 512
DV = 128
DN = 128
DQ = 192
EPS = 1e-6
SCALE = DQ ** -0.5
BSZ, KVLEN = 32, 4096

N_CORES = 8
BPC = BSZ // N_CORES          # sequences per core
KT = KVLEN // 128             # 32 k-tiles of 128 positions
NQ = 4                        # score quarters (psum-sized chunks of k)
KQ = KVLEN // NQ              # 1024 score columns per quarter
TQ = KQ // 128                # 8 k-tiles per quarter

BF16 = mybir.dt.bfloat16
F32 = mybir.dt.float32
AF = mybir.ActivationFunctionType
ALU = mybir.AluOpType


def _emit(tc, t):
    nc = tc.nc

    with tc.tile_pool(name="cpool", bufs=1) as cpool, \
         tc.tile_pool(name="wpool", bufs=2) as wpool, \
         tc.tile_pool(name="cachepool", bufs=2) as cachepool:

        # ---------------- constants / persistent small tensors ----------------
        id_bf = cpool.tile([128, 128], BF16)
        make_identity(nc, id_bf)
        id_f32 = cpool.tile([128, 128], F32)
        make_identity(nc, id_f32)

        hidT_sb = cpool.tile([128, 16 * BPC], BF16)
        nc.sync.dma_start(hidT_sb, t["hidT"][:, :])
        wukt_sb = cpool.tile([128, NH * DC], BF16)
        nc.sync.dma_start(wukt_sb, t["wukt"][:, :])
        wuv_sb = cpool.tile([128, NH * 4 * DV], BF16)
        nc.sync.dma_start(wuv_sb, t["wuv"][:, :])
        cosT_sb = cpool.tile([DR, 1], F32)
        nc.sync.dma_start(cosT_sb, t["cosT"][:, :])
        sinT_sb = cpool.tile([DR, 1], F32)
        nc.sync.dma_start(sinT_sb, t["sinT"][:, :])
        lnw_sb = cpool.tile([BPC, DC], F32)
        nc.sync.dma_start(lnw_sb, t["lnw"][:, :])

        qabsT = cpool.tile([128, 4 * NH * BPC], BF16)   # [c%128, (j, h, b)]
        qpeT_f32 = cpool.tile([DR, NH * BPC], F32)      # [r, (h, b)]
        qpeT_b16 = cpool.tile([DR, NH * BPC], BF16)
        cn_b16 = cpool.tile([BPC, DC], BF16)            # c_norm rows (natural fixup)
        cnT = cpool.tile([128, 4 * BPC], BF16)          # c_norm cols [(j, b)]
        kpenT_b16 = cpool.tile([DR, BPC], BF16)         # roped new k_pe cols

        def rope_cols(x_f32, out_b16, pool, nm):
            # rope along the partition (r) axis of [64, n]; cos/sin per-partition
            n = x_f32.shape[-1]
            rot = pool.tile([DR, n], F32, tag=f"rot{nm}", name=f"rot{nm}")
            nc.scalar.mul(rot[0:DR // 2, :], x_f32[DR // 2:DR, :], -1.0)
            nc.scalar.copy(rot[DR // 2:DR, :], x_f32[0:DR // 2, :])
            t1 = pool.tile([DR, n], F32, tag=f"t1{nm}", name=f"t1{nm}")
            nc.vector.tensor_scalar_mul(t1, x_f32, cosT_sb)
            nc.vector.tensor_scalar_mul(rot, rot, sinT_sb)
            nc.vector.tensor_add(t1, t1, rot)
            nc.vector.tensor_copy(out_b16, t1)

        # ---------------- stage A: projections ----------------
        with tc.tile_pool(name="psA", bufs=1, space="PSUM") as psA:
            # q = hidden @ W_UQR  -> [4, 3072] (psum)
            q_ps = psA.tile([BPC, NH * DQ], F32, tag="big", bufs=1)
            for i in range(16):
                wq = wpool.tile([128, NH * DQ], BF16, tag="wuqr", bufs=3, name="wq")
                nc.sync.dma_start(wq, t["wuqr"][i * 128:(i + 1) * 128, :])
                lhsT = hidT_sb[:, i * BPC:(i + 1) * BPC]
                for n in range(6):
                    nc.tensor.matmul(q_ps[:, n * 512:(n + 1) * 512], lhsT,
                                     wq[:, n * 512:(n + 1) * 512],
                                     start=(i == 0), stop=(i == 15))
            q_sb = cpool.tile([BPC, NH * DQ], F32)
            nc.scalar.copy(q_sb, q_ps)

            # latent = hidden @ W_kva -> [4, 576]
            lat_ps = psA.tile([BPC, DC + DR], F32, tag="big", bufs=1, name="lat_ps")
            for i in range(16):
                wk = wpool.tile([128, DC + DR], BF16, tag="wkva", bufs=3, name="wk")
                nc.sync.dma_start(wk, t["wkva"][i * 128:(i + 1) * 128, :])
                lhsT = hidT_sb[:, i * BPC:(i + 1) * BPC]
                nc.tensor.matmul(lat_ps[:, 0:DC], lhsT, wk[:, 0:DC],
                                 start=(i == 0), stop=(i == 15))
                nc.tensor.matmul(lat_ps[:, DC:DC + DR], lhsT, wk[:, DC:DC + DR],
                                 start=(i == 0), stop=(i == 15))
            lat_sb = cpool.tile([BPC, DC + DR], F32)
            nc.scalar.copy(lat_sb, lat_ps)

            # rms_norm(latent[:, :512]) * ln_w
            sq = cpool.tile([BPC, DC], F32)
            ssq = cpool.tile([BPC, 1], F32)
            nc.scalar.activation(sq, lat_sb[:, :DC], AF.Square, accum_out=ssq)
            eps_sb = cpool.tile([BPC, 1], F32)
            nc.vector.memset(eps_sb, EPS)
            stdv = cpool.tile([BPC, 1], F32)
            nc.scalar.activation(stdv, ssq, AF.Sqrt, scale=1.0 / DC, bias=eps_sb)
            rinv = cpool.tile([BPC, 1], F32)
            nc.vector.reciprocal(rinv, stdv)
            cn = cpool.tile([BPC, DC], F32)
            nc.vector.tensor_scalar_mul(cn, lat_sb[:, :DC], rinv)
            nc.vector.tensor_mul(cn, cn, lnw_sb)
            nc.vector.tensor_copy(cn_b16, cn)
            for j in range(4):
                tp = psA.tile([128, BPC], F32, tag="small", bufs=2, name="tp")
                nc.tensor.transpose(tp, cn[:, j * 128:(j + 1) * 128],
                                    id_f32[0:BPC, 0:BPC])
                nc.vector.tensor_copy(cnT[:, j * BPC:(j + 1) * BPC], tp)

            # new-token k_pe: transpose then rope (cols)
            kpT = psA.tile([DR, BPC], F32, tag="small", bufs=2, name="kpT")
            nc.tensor.transpose(kpT, lat_sb[:, DC:DC + DR], id_f32[0:BPC, 0:BPC])
            kpe_f32 = cpool.tile([DR, BPC], F32)
            nc.vector.tensor_copy(kpe_f32, kpT)
            rope_cols(kpe_f32, kpenT_b16, cpool, "k")

            # per-head q transposes + W_UK absorption
            for h in range(NH):
                tpn = psA.tile([128, BPC], F32, tag="small", bufs=2, name="tpn")
                nc.tensor.transpose(tpn, q_sb[:, h * DQ:h * DQ + DN],
                                    id_f32[0:BPC, 0:BPC])
                qnT = wpool.tile([128, BPC], BF16, tag="qnT", bufs=2, name="qnT")
                nc.vector.tensor_copy(qnT, tpn)
                aps = psA.tile([BPC, DC], F32, tag="small", bufs=2, name="aps")
                nc.tensor.matmul(aps, qnT, wukt_sb[:, h * DC:(h + 1) * DC],
                                 start=True, stop=True)
                qabs_sb = wpool.tile([BPC, DC], F32, tag="qabs_sb", bufs=2,
                                     name="qabs_sb")
                nc.scalar.copy(qabs_sb, aps)
                qaview = qabsT.rearrange("p (j h b) -> p j h b", j=4, h=NH, b=BPC)
                for j in range(4):
                    tpa = psA.tile([128, BPC], F32, tag="small", bufs=2, name="tpa")
                    nc.tensor.transpose(tpa, qabs_sb[:, j * 128:(j + 1) * 128],
                                        id_f32[0:BPC, 0:BPC])
                    nc.vector.tensor_copy(qaview[:, j, h, :], tpa)
                tpp = psA.tile([DR, BPC], F32, tag="small", bufs=2, name="tpp")
                nc.tensor.transpose(tpp, q_sb[:, h * DQ + DN:(h + 1) * DQ],
                                    id_f32[0:BPC, 0:BPC])
                nc.vector.tensor_copy(qpeT_f32[:, h * BPC:(h + 1) * BPC], tpp)
            rope_cols(qpeT_f32, qpeT_b16, cpool, "q")

        qa = qabsT.rearrange("p (j h b) -> p j h b", j=4, h=NH, b=BPC)
        qp = qpeT_b16.rearrange("p (h b) -> p h b", h=NH, b=BPC)

        # ---------------- stage B: flash attention per sequence ----------------
        attn_sbs = []
        with tc.tile_pool(name="psB", bufs=1, space="PSUM") as psB:
            for b in range(BPC):
                natv = t["ckv_nat"][b].rearrange("(g t p) c -> g t p c",
                                                 p=128, t=TQ)
                ckvTv = t["ckv_t"][b]
                kpeTv = t["kpe_t"][b]

                nats = []
                for g in range(NQ):
                    nat = cachepool.tile([128, TQ * DC], BF16, tag="nat", bufs=3,
                                         name="nat")
                    nc.scalar.dma_start(nat.rearrange("p (t c) -> p t c", t=TQ),
                                        natv[g].rearrange("t p c -> p t c"))
                    nats.append(nat)
                # place normed new-token latent in the last cache slot (row 127
                # of the last k-tile) — SBUF->SBUF DMA for the cross-partition move
                nc.sync.dma_start(nats[NQ - 1][127:128, (TQ - 1) * DC:TQ * DC],
                                  cn_b16[b:b + 1, :])

                probs = cachepool.tile([NH, KVLEN], BF16, tag="probs", bufs=2,
                                       name="probs")
                probsT = cachepool.tile([128, KT * NH], BF16, tag="probsT", bufs=2,
                                        name="probsT")
                den = wpool.tile([NH, NQ], F32, tag="den", bufs=2, name="den")
                attn_ps = psB.tile([NH, DC], F32, tag="attn", bufs=2, name="attn_ps")

                for q in range(NQ):
                    ckvT_tiles = []
                    for j in range(4):
                        ct = cachepool.tile([128, KQ], BF16, tag="ckvT", bufs=8,
                                            name="ct")
                        nc.scalar.dma_start(
                            ct, ckvTv[j * 128:(j + 1) * 128, q * KQ:(q + 1) * KQ])
                        ckvT_tiles.append(ct)
                    kt_ = cachepool.tile([DR, KQ], BF16, tag="kpeT", bufs=2,
                                         name="kt_")
                    nc.scalar.dma_start(kt_, kpeTv[:, q * KQ:(q + 1) * KQ])
                    if q == NQ - 1:
                        for j in range(4):
                            nc.vector.tensor_copy(
                                ckvT_tiles[j][:, KQ - 1:KQ],
                                cnT[:, j * BPC + b:j * BPC + b + 1])
                        nc.vector.tensor_copy(kt_[:, KQ - 1:KQ],
                                              kpenT_b16[:, b:b + 1])

                    sc = psB.tile([NH, KQ], F32, tag="scores", bufs=2, name="sc")
                    for half in range(2):
                        csl = slice(half * 512, (half + 1) * 512)
                        for j in range(4):
                            nc.tensor.matmul(sc[:, csl], qa[:, j, :, b],
                                             ckvT_tiles[j][:, csl],
                                             start=(j == 0), stop=False)
                        nc.tensor.matmul(sc[:, csl], qp[:, :, b], kt_[:, csl],
                                         start=False, stop=True)
                    # exp (softmax numerator) + running denominator
                    nc.scalar.activation(probs[:, q * KQ:(q + 1) * KQ], sc, AF.Exp,
                                         scale=SCALE, accum_out=den[:, q:q + 1])
                    pT = psB.tile([128, TQ * NH], BF16, tag="pT", bufs=2, name="pT")
                    for tl in range(TQ):
                        nc.tensor.transpose(
                            pT[:, tl * NH:(tl + 1) * NH],
                            probs[:, q * KQ + tl * 128:q * KQ + (tl + 1) * 128],
                            id_bf[0:NH, 0:NH])
                    nc.vector.tensor_copy(
                        probsT[:, q * TQ * NH:(q + 1) * TQ * NH], pT)
                    for tl in range(TQ):
                        tg = q * TQ + tl
                        nc.tensor.matmul(attn_ps,
                                         probsT[:, tg * NH:(tg + 1) * NH],
                                         nats[q][:, tl * DC:(tl + 1) * DC],
                                         start=(tg == 0), stop=(tg == KT - 1))

                dsum = wpool.tile([NH, 1], F32, tag="dsum", bufs=2, name="dsum")
                nc.vector.tensor_reduce(dsum, den, axis=mybir.AxisListType.X,
                                        op=ALU.add)
                rin = wpool.tile([NH, 1], F32, tag="rin", bufs=2, name="rin")
                nc.vector.reciprocal(rin, dsum)
                attn_sb = cpool.tile([NH, DC], F32, tag=f"attn{b}",
                                     name=f"attn_sb{b}")
                nc.scalar.activation(attn_sb, attn_ps, AF.Copy, scale=rin)
                attn_sbs.append(attn_sb)

        # ---------------- stage C: W_UV absorption + output projection ----------------
        with tc.tile_pool(name="psC", bufs=1, space="PSUM") as psC:
            attnT = cpool.tile([128, 4 * NH * BPC], BF16)   # [c%128, (j, h, b)]
            av = attnT.rearrange("p (j h b) -> p j h b", j=4, h=NH, b=BPC)
            for b in range(BPC):
                for j in range(4):
                    ap_ = psC.tile([128, NH], F32, tag="att", bufs=2, name="ap_")
                    nc.tensor.transpose(ap_, attn_sbs[b][:, j * 128:(j + 1) * 128],
                                        id_f32[0:NH, 0:NH])
                    nc.vector.tensor_copy(av[:, j, :, b], ap_)

            vT = cpool.tile([128, NH * BPC], BF16)          # [dv, (h, b)]
            wuv_v = wuv_sb.rearrange("p (h j v) -> p h j v", h=NH, j=4, v=DV)
            for h in range(NH):
                vps = psC.tile([128, BPC], F32, tag="vt", bufs=2, name="vps")
                for j in range(4):
                    nc.tensor.matmul(vps, wuv_v[:, h, j, :], av[:, j, h, :],
                                     start=(j == 0), stop=(j == 3))
                nc.vector.tensor_copy(vT[:, h * BPC:(h + 1) * BPC], vps)

            y_ps = [psC.tile([BPC, 512], F32, tag="y", bufs=4, name=f"y{n}")
                    for n in range(4)]
            for h in range(NH):
                wo_t = wpool.tile([128, H], BF16, tag="wo", bufs=3, name="wo_t")
                nc.sync.dma_start(wo_t, t["wo"][h * DV:(h + 1) * DV, :])
                for n in range(4):
                    nc.tensor.matmul(y_ps[n], vT[:, h * BPC:(h + 1) * BPC],
                                     wo_t[:, n * 512:(n + 1) * 512],
                                     start=(h == 0), stop=(h == NH - 1))
            y_sb = cpool.tile([BPC, H], F32)
            for n in range(4):
                nc.scalar.copy(y_sb[:, n * 512:(n + 1) * 512], y_ps[n])
            nc.sync.dma_start(t["out"][:, :], y_sb)


def build_module(debug=False):
    nc = bacc.Bacc("TRN2", target_bir_lowering=False, debug=debug,
                   num_devices=N_CORES)
    t = {}
    t["ckv_nat"] = nc.dram_tensor("ckv_nat", [BPC, KVLEN, DC], BF16,
                                  kind="ExternalInput")
    t["ckv_t"] = nc.dram_tensor("ckv_t", [BPC, DC, KVLEN], BF16,
                                kind="ExternalInput")
    t["kpe_t"] = nc.dram_tensor("kpe_t", [BPC, DR, KVLEN], BF16,
                                kind="ExternalInput")
    t["hidT"] = nc.dram_tensor("hidT", [128, 16 * BPC], BF16,
                               kind="ExternalInput")
    t["wuqr"] = nc.dram_tensor("wuqr", [H, NH * DQ], BF16, kind="ExternalInput")
    t["wukt"] = nc.dram_tensor("wukt", [128, NH * DC], BF16,
                               kind="ExternalInput")
    t["wkva"] = nc.dram_tensor("wkva", [H, DC + DR], BF16, kind="ExternalInput")
    t["wuv"] = nc.dram_tensor("wuv", [128, NH * 4 * DV], BF16,
                              kind="ExternalInput")
    t["wo"] = nc.dram_tensor("wo", [NH * DV, H], BF16, kind="ExternalInput")
    t["lnw"] = nc.dram_tensor("lnw", [BPC, DC], F32, kind="ExternalInput")
    t["cosT"] = nc.dram_tensor("cosT", [DR, 1], F32, kind="ExternalInput")
    t["sinT"] = nc.dram_tensor("sinT", [DR, 1], F32, kind="ExternalInput")
    t["out"] = nc.dram_tensor("out", [BPC, H], F32, kind="ExternalOutput")

    with tile.TileContext(nc) as tc:
        _emit(tc, t)
    nc.compile()
    return nc


def prep_inputs(hidden_states, compressed_kv_normed_cache, k_pe_cache,
                W_UQR, W_kva, ln_w, W_UK, W_UV, W_O, cos, sin):
    """Host-side layout/dtype prep + per-core sharding. Returns in_maps."""
    bf16 = ml_dtypes.bfloat16
    f32 = np.float32

    wuqr = np.ascontiguousarray(np.asarray(W_UQR)).astype(bf16)
    # W_UK [h, c, d] -> [d, (h c)]
    wukt = np.asarray(W_UK).transpose(2, 0, 1).reshape(128, NH * DC).astype(bf16)
    wukt = np.ascontiguousarray(wukt)
    wkva = np.ascontiguousarray(np.asarray(W_kva)).astype(bf16)
    # W_UV [h, c, v] -> [c%128, (h, j, v)]
    wuv = np.asarray(W_UV).reshape(NH, 4, 128, DV).transpose(2, 0, 1, 3)
    wuv = np.ascontiguousarray(wuv.reshape(128, NH * 4 * DV)).astype(bf16)
    wo = np.ascontiguousarray(np.asarray(W_O)).astype(bf16)
    lnw = np.tile(np.asarray(ln_w, dtype=f32)[None, :], (BPC, 1))
    cosT = np.ascontiguousarray(np.asarray(cos, dtype=f32).reshape(1, DR).T)
    sinT = np.ascontiguousarray(np.asarray(sin, dtype=f32).reshape(1, DR).T)

    ckv = np.asarray(compressed_kv_normed_cache)
    kpe = np.asarray(k_pe_cache)
    hs = np.asarray(hidden_states)

    ckv_nat = ckv.astype(bf16)                                   # [32, k, c]
    ckv_t = ckv.transpose(0, 2, 1).astype(bf16)                  # [32, c, k]
    ckv_t = np.ascontiguousarray(ckv_t)
    kpe_t = np.ascontiguousarray(kpe.transpose(0, 2, 1).astype(bf16))

    in_maps = []
    for c in range(N_CORES):
        sl = slice(c * BPC, (c + 1) * BPC)
        hidT_c = hs[sl].T.reshape(16, 128, BPC).transpose(1, 0, 2)
        hidT_c = np.ascontiguousarray(hidT_c.reshape(128, 16 * BPC)).astype(bf16)
        in_maps.append({
            "ckv_nat": np.ascontiguousarray(ckv_nat[sl]),
            "ckv_t": np.ascontiguousarray(ckv_t[sl]),
            "kpe_t": np.ascontiguousarray(kpe_t[sl]),
            "hidT": hidT_c,
            "wuqr": wuqr, "wukt": wukt, "wkva": wkva, "wuv": wuv, "wo": wo,
            "lnw": lnw.astype(f32), "cosT": cosT.astype(f32),
            "sinT": sinT.astype(f32),
        })
    return in_maps


_MODULE = None


def _get_module():
    global _MODULE
    if _MODULE is None:
        _MODULE = build_module()
    return _MODULE


def kernel(**inputs):
    nc = _get_module()
    in_maps = prep_inputs(**inputs)
    res = run_bass_kernel_spmd(nc, in_maps, core_ids=list(range(N_CORES)))
    out = np.concatenate([r["out"] for r in res.results], axis=0)
    return np.ascontiguousarray(out.astype(np.float32))


# revision 21
# speedup vs baseline: 314.2433x; 314.2433x over previous
"""DeepSeek-V2-Lite matrix-absorbed MLA decode on 8 Trainium2 NeuronCores.

Sharding: attention is data-parallel over batch (4 sequences + their KV cache
slices per core). The query/latent projections are tensor-parallel: each core
computes its 2 heads (W_UQR/W_UK column shard) and a W_kva row-shard partial
for ALL 32 sequences, then an AllToAll (q) + ReduceScatter (latent) hand every
core all 16 heads for its own 4 sequences. W_UV/W_O stay replicated (the
output-side collectives would sit on the critical-path tail).

Host-side input prep casts the KV caches and weights to bf16 and ships the
compressed-KV cache in both natural [k, c] and transposed [c, k] layouts so
both attention matmuls stream through the PE with no on-device transposes of
large tensors. Attention is a single flash pass: softmax skips the max
subtraction (|scores*scale| <= ~4 for this problem family, exp stays finite in
fp32) and the denominator comes from the Exp activation's accum_out.
"""

import sys

import numpy as np
import ml_dtypes

for _p in ("/opt/trn_rl_repo",):
    if _p not in sys.path:
        sys.path.insert(0, _p)

import concourse.bass as bass  # noqa: E402
import concourse.mybir as mybir  # noqa: E402
import concourse.tile as tile  # noqa: E402
from concourse import bacc  # noqa: E402
from concourse.bass_utils import run_bass_kernel_spmd  # noqa: E402
from concourse.masks import make_identity  # noqa: E402

# Problem constants (hardcoded per harness contract).
H = 2048
NH = 16
DR = 64
DC = 512
DV = 128
DN = 128
DQ = 192
EPS = 1e-6
SCALE = DQ ** -0.5
BSZ, KVLEN = 32, 4096

N_CORES = 8
BPC = BSZ // N_CORES          # sequences per core
KT = KVLEN // 128             # 32 k-tiles of 128 positions
NQ = 4                        # score quarters (psum-sized chunks of k)
KQ = KVLEN // NQ              # 1024 score columns per quarter
TQ = KQ // 128                # 8 k-tiles per quarter

BF16 = mybir.dt.bfloat16
F32 = mybir.dt.float32
AF = mybir.ActivationFunctionType
ALU = mybir.AluOpType


def _emit(tc, t):
    nc = tc.nc

    with tc.tile_pool(name="cpool", bufs=1) as cpool, \
         tc.tile_pool(name="wpool", bufs=2) as wpool, \
         tc.tile_pool(name="cachepool", bufs=2) as cachepool:

        # ---------------- constants / persistent small tensors ----------------
        id_bf = cpool.tile([128, 128], BF16)
        make_identity(nc, id_bf)
        id_f32 = cpool.tile([128, 128], F32)
        make_identity(nc, id_f32)

        hidT_sb = cpool.tile([128, 16 * BSZ], BF16)     # all 32 sequences
        nc.sync.dma_start(hidT_sb, t["hidT"][:, :])
        hidkva_sb = cpool.tile([128, 2 * BSZ], BF16)    # hid chunks for W_kva slice
        nc.sync.dma_start(hidkva_sb, t["hidT_kva"][:, :])
        wukt_sb = cpool.tile([128, 2 * DC], BF16)       # this core's 2 heads
        nc.sync.dma_start(wukt_sb, t["wukt"][:, :])
        wuv_sb = cpool.tile([128, NH * 4 * DV], BF16)
        nc.sync.dma_start(wuv_sb, t["wuv"][:, :])
        cosT_sb = cpool.tile([DR, 1], F32)
        nc.sync.dma_start(cosT_sb, t["cosT"][:, :])
        sinT_sb = cpool.tile([DR, 1], F32)
        nc.sync.dma_start(sinT_sb, t["sinT"][:, :])
        lnw_sb = cpool.tile([BPC, DC], F32)
        nc.sync.dma_start(lnw_sb, t["lnw"][:, :])

        qabsT = cpool.tile([128, N_CORES * 4 * BPC * 2], BF16)  # [p,(s,j,bl,hl)]
        qpeT_b16 = cpool.tile([DR, N_CORES * BPC * 2], BF16)    # [r,(s,bl,hl)]
        cn_b16 = cpool.tile([BPC, DC], BF16)            # c_norm rows (natural fixup)
        cnT = cpool.tile([128, 4 * BPC], BF16)          # c_norm cols [(j, b)]
        kpenT_b16 = cpool.tile([DR, BPC], BF16)         # roped new k_pe cols

        def rope_cols(x_f32, out_b16, pool, nm):
            # rope along the partition (r) axis of [64, n]; cos/sin per-partition
            n = x_f32.shape[-1]
            rot = pool.tile([DR, n], F32, tag=f"rot{nm}", name=f"rot{nm}")
            nc.scalar.mul(rot[0:DR // 2, :], x_f32[DR // 2:DR, :], -1.0)
            nc.scalar.copy(rot[DR // 2:DR, :], x_f32[0:DR // 2, :])
            t1 = pool.tile([DR, n], F32, tag=f"t1{nm}", name=f"t1{nm}")
            nc.vector.tensor_scalar_mul(t1, x_f32, cosT_sb)
            nc.vector.tensor_scalar_mul(rot, rot, sinT_sb)
            nc.vector.tensor_add(t1, t1, rot)
            nc.vector.tensor_copy(out_b16, t1)

        # ---------------- stage A: sharded projections + exchange ----------------
        RG = [list(range(N_CORES))]
        with tc.tile_pool(name="psA", bufs=1, space="PSUM") as psA, \
             tc.tile_pool(name="dpool", bufs=1, space="DRAM") as dpool:
            # q for this core's 2 heads, ALL 32 sequences
            wuqr_sb = cpool.tile([128, 16 * 2 * DQ], BF16)
            nc.sync.dma_start(wuqr_sb.rearrange("p (i n) -> p i n", i=16),
                              t["wuqr"].rearrange("(i p) n -> p i n", p=128))
            q_ps = psA.tile([BSZ, 2 * DQ], F32, tag="qps", bufs=1)
            for i in range(16):
                nc.tensor.matmul(q_ps, hidT_sb[:, i * BSZ:(i + 1) * BSZ],
                                 wuqr_sb[:, i * 2 * DQ:(i + 1) * 2 * DQ],
                                 start=(i == 0), stop=(i == 15))
            q_sb = cpool.tile([BSZ, 2 * DQ], F32)
            nc.scalar.copy(q_sb, q_ps)

            # partial latent from this core's W_kva row-slice, ReduceScatter(add)
            wkva_sb = cpool.tile([128, 2 * (DC + DR)], BF16)
            nc.sync.dma_start(wkva_sb.rearrange("p (c n) -> p c n", c=2),
                              t["wkva"].rearrange("(c p) n -> p c n", p=128))
            lat_ps = psA.tile([BSZ, DC + DR], F32, tag="latps", bufs=1)
            for c in range(2):
                lhsT = hidkva_sb[:, c * BSZ:(c + 1) * BSZ]
                w0 = c * (DC + DR)
                nc.tensor.matmul(lat_ps[:, 0:DC], lhsT, wkva_sb[:, w0:w0 + DC],
                                 start=(c == 0), stop=(c == 1))
                nc.tensor.matmul(lat_ps[:, DC:DC + DR], lhsT,
                                 wkva_sb[:, w0 + DC:w0 + DC + DR],
                                 start=(c == 0), stop=(c == 1))
            latp_sb = wpool.tile([BSZ, DC + DR], F32, tag="latp", name="latp_sb")
            nc.scalar.copy(latp_sb, lat_ps)
            latp_d = dpool.tile([BSZ, DC + DR], F32, name="latp_d")
            nc.sync.dma_start(latp_d, latp_sb)
            latr_d = dpool.tile([BPC, DC + DR], F32, name="latr_d")
            nc.gpsimd.collective_compute("ReduceScatter", ALU.add, RG,
                                         [latp_d[:, :]], [latr_d[:, :]])
            lat_sb = cpool.tile([BPC, DC + DR], F32)
            nc.sync.dma_start(lat_sb, latr_d[:, :])

            # rms_norm(latent[:, :512]) * ln_w
            sq = cpool.tile([BPC, DC], F32)
            ssq = cpool.tile([BPC, 1], F32)
            nc.scalar.activation(sq, lat_sb[:, :DC], AF.Square, accum_out=ssq)
            eps_sb = cpool.tile([BPC, 1], F32)
            nc.vector.memset(eps_sb, EPS)
            stdv = cpool.tile([BPC, 1], F32)
            nc.scalar.activation(stdv, ssq, AF.Sqrt, scale=1.0 / DC, bias=eps_sb)
            rinv = cpool.tile([BPC, 1], F32)
            nc.vector.reciprocal(rinv, stdv)
            cn = cpool.tile([BPC, DC], F32)
            nc.vector.tensor_scalar_mul(cn, lat_sb[:, :DC], rinv)
            nc.vector.tensor_mul(cn, cn, lnw_sb)
            nc.vector.tensor_copy(cn_b16, cn)
            for j in range(4):
                tp = psA.tile([128, BPC], F32, tag="small", bufs=2, name="tp")
                nc.tensor.transpose(tp, cn[:, j * 128:(j + 1) * 128],
                                    id_f32[0:BPC, 0:BPC])
                nc.vector.tensor_copy(cnT[:, j * BPC:(j + 1) * BPC], tp)

            # new-token k_pe: transpose then rope (cols)
            kpT = psA.tile([DR, BPC], F32, tag="small", bufs=2, name="kpT")
            nc.tensor.transpose(kpT, lat_sb[:, DC:DC + DR], id_f32[0:BPC, 0:BPC])
            kpe_f32 = cpool.tile([DR, BPC], F32)
            nc.vector.tensor_copy(kpe_f32, kpT)
            rope_cols(kpe_f32, kpenT_b16, cpool, "k")

            # this core's 2 heads: transposes + W_UK absorption -> send layout
            qsend_sb = cpool.tile([128, N_CORES * 4 * BPC * 2], BF16)
            qpesend_sb = cpool.tile([DR, N_CORES * BPC * 2], BF16)
            qpe2_f32 = cpool.tile([DR, 2 * BSZ], F32)
            qs_v = qsend_sb.rearrange("p (d j bl hl) -> p d j bl hl",
                                      d=N_CORES, j=4, bl=BPC, hl=2)
            for hl in range(2):
                tpn = psA.tile([128, BSZ], F32, tag="small", bufs=2, name="tpn")
                nc.tensor.transpose(tpn, q_sb[:, hl * DQ:hl * DQ + DN],
                                    id_f32[0:BSZ, 0:BSZ])
                qnT = wpool.tile([128, BSZ], BF16, tag="qnT", bufs=2, name="qnT")
                nc.vector.tensor_copy(qnT, tpn)
                aps = psA.tile([BSZ, DC], F32, tag="small", bufs=2, name="aps")
                nc.tensor.matmul(aps, qnT, wukt_sb[:, hl * DC:(hl + 1) * DC],
                                 start=True, stop=True)
                qabs_sb = wpool.tile([BSZ, DC], F32, tag="qabs_sb", bufs=2,
                                     name="qabs_sb")
                nc.scalar.copy(qabs_sb, aps)
                for j in range(4):
                    tpa = psA.tile([128, BSZ], F32, tag="small", bufs=2, name="tpa")
                    nc.tensor.transpose(tpa, qabs_sb[:, j * 128:(j + 1) * 128],
                                        id_f32[0:BSZ, 0:BSZ])
                    nc.vector.tensor_copy(
                        qs_v[:, :, j, :, hl],
                        tpa.rearrange("p (d bl) -> p d bl", d=N_CORES))
                tpp = psA.tile([DR, BSZ], F32, tag="small", bufs=2, name="tpp")
                nc.tensor.transpose(tpp, q_sb[:, hl * DQ + DN:(hl + 1) * DQ],
                                    id_f32[0:BSZ, 0:BSZ])
                nc.vector.tensor_copy(qpe2_f32[:, hl * BSZ:(hl + 1) * BSZ], tpp)
            qpe2_roped = cpool.tile([DR, 2 * BSZ], F32)
            rope_cols(qpe2_f32, qpe2_roped, cpool, "q")
            qpv = qpesend_sb.rearrange("r (d bl hl) -> r d bl hl",
                                       d=N_CORES, bl=BPC, hl=2)
            for hl in range(2):
                nc.vector.tensor_copy(
                    qpv[:, :, :, hl],
                    qpe2_roped[:, hl * BSZ:(hl + 1) * BSZ].rearrange(
                        "r (d bl) -> r d bl", d=N_CORES))

            # AllToAll: each core ends with all 16 heads for its 4 sequences
            QCH = 4 * BPC * 2 * 128 + BPC * 2 * DR     # per-dest chunk (elems)
            QA = 4 * BPC * 2 * 128                     # qabs region size
            qsend_d = dpool.tile([N_CORES, QCH], BF16, name="qsend_d")
            nc.sync.dma_start(
                qsend_d[:, 0:QA].rearrange("d (p c) -> p d c", p=128),
                qsend_sb.rearrange("p (d c) -> p d c", d=N_CORES))
            nc.sync.dma_start(
                qsend_d[:, QA:QCH].rearrange("d (r c) -> r d c", r=DR),
                qpesend_sb.rearrange("r (d c) -> r d c", d=N_CORES))
            qrecv_d = dpool.tile([N_CORES, QCH], BF16, name="qrecv_d")
            nc.gpsimd.collective_compute("AllToAll", ALU.bypass, RG,
                                         [qsend_d[:, :]], [qrecv_d[:, :]])
            # land src-major (simple 3-dim DMA), then one DVE copy reorders so
            # the 16 head columns (src, hl) are contiguous per (j, bl) — the
            # scores lhsT slices must be plain 2-D APs for walrus
            qabs_raw = cpool.tile([128, N_CORES * 4 * BPC * 2], BF16)
            nc.sync.dma_start(
                qabs_raw.rearrange("p (s c) -> p s c", s=N_CORES),
                qrecv_d[:, 0:QA].rearrange("s (p c) -> p s c", p=128))
            qpe_raw = cpool.tile([DR, N_CORES * BPC * 2], BF16)
            nc.sync.dma_start(
                qpe_raw.rearrange("r (s c) -> r s c", s=N_CORES),
                qrecv_d[:, QA:QCH].rearrange("s (r c) -> r s c", r=DR))
            nc.vector.tensor_copy(
                qabsT.rearrange("p (j bl s hl) -> p s j bl hl",
                                j=4, bl=BPC, s=N_CORES),
                qabs_raw.rearrange("p (s j bl hl) -> p s j bl hl",
                                   s=N_CORES, j=4, bl=BPC))
            nc.vector.tensor_copy(
                qpeT_b16.rearrange("r (bl s hl) -> r s bl hl",
                                   bl=BPC, s=N_CORES),
                qpe_raw.rearrange("r (s bl hl) -> r s bl hl",
                                  s=N_CORES, bl=BPC))

        qa = qabsT.rearrange("p (j bl shl) -> p j bl shl", j=4, bl=BPC)
        qp = qpeT_b16.rearrange("r (bl shl) -> r bl shl", bl=BPC)

        # ---------------- stage B: flash attention per sequence ----------------
        attn_sbs = []
        with tc.tile_pool(name="psB", bufs=1, space="PSUM") as psB:
            for b in range(BPC):
                natv = t["ckv_nat"][b].rearrange("(g t p) c -> g t p c",
                                                 p=128, t=TQ)
                # ckv_t [512, 4096] viewed [p(c%128), j, k] for packed loads
                ckvTj = t["ckv_t"][b].rearrange("(j p) k -> p j k", p=128)
                kpeTv = t["kpe_t"][b]

                nats = []
                for g in range(NQ):
                    nat = cachepool.tile([128, TQ * DC], BF16, tag="nat", bufs=3,
                                         name="nat")
                    nc.scalar.dma_start(nat.rearrange("p (t c) -> p t c", t=TQ),
                                        natv[g].rearrange("t p c -> p t c"))
                    nats.append(nat)
                kt_ = cachepool.tile([DR, KVLEN], BF16, tag="kpeT", bufs=2,
                                     name="kt_")
                nc.scalar.dma_start(kt_, kpeTv[:, :])
                nc.vector.tensor_copy(kt_[:, KVLEN - 1:KVLEN],
                                      kpenT_b16[:, b:b + 1])
                # place normed new-token latent in the last cache slot (row 127
                # of the last k-tile) — SBUF->SBUF DMA for the cross-partition move
                nc.sync.dma_start(nats[NQ - 1][127:128, (TQ - 1) * DC:TQ * DC],
                                  cn_b16[b:b + 1, :])

                probs = cachepool.tile([NH, KVLEN], BF16, tag="probs", bufs=2,
                                       name="probs")
                probsT = cachepool.tile([128, KT * NH], BF16, tag="probsT", bufs=2,
                                        name="probsT")
                den = wpool.tile([NH, NQ], F32, tag="den", bufs=2, name="den")
                attn_ps = psB.tile([NH, DC], F32, tag="attn", bufs=2, name="attn_ps")

                for q in range(NQ):
                    ct = cachepool.tile([128, 4 * KQ], BF16, tag="ckvT", bufs=2,
                                        name="ct")
                    ctv = ct.rearrange("p (j k) -> p j k", j=4)
                    nc.scalar.dma_start(ctv, ckvTj[:, :, q * KQ:(q + 1) * KQ])
                    if q == NQ - 1:
                        for j in range(4):
                            nc.vector.tensor_copy(
                                ctv[:, j, KQ - 1:KQ],
                                cnT[:, j * BPC + b:j * BPC + b + 1])

                    sc = psB.tile([NH, KQ], F32, tag="scores", bufs=2, name="sc")
                    for half in range(2):
                        csl = slice(half * 512, (half + 1) * 512)
                        for j in range(4):
                            nc.tensor.matmul(sc[:, csl], qa[:, j, b, :],
                                             ctv[:, j, csl],
                                             start=(j == 0), stop=False)
                        nc.tensor.matmul(sc[:, csl], qp[:, b, :],
                                         kt_[:, q * KQ:(q + 1) * KQ][:, csl],
                                         start=False, stop=True)
                    # exp (softmax numerator) + running denominator
                    nc.scalar.activation(probs[:, q * KQ:(q + 1) * KQ], sc, AF.Exp,
                                         scale=SCALE, accum_out=den[:, q:q + 1])
                    pT = psB.tile([128, TQ * NH], BF16, tag="pT", bufs=2, name="pT")
                    for tl in range(TQ):
                        nc.tensor.transpose(
                            pT[:, tl * NH:(tl + 1) * NH],
                            probs[:, q * KQ + tl * 128:q * KQ + (tl + 1) * 128],
                            id_bf[0:NH, 0:NH])
                    nc.vector.tensor_copy(
                        probsT[:, q * TQ * NH:(q + 1) * TQ * NH], pT)
                    for tl in range(TQ):
                        tg = q * TQ + tl
                        nc.tensor.matmul(attn_ps,
                                         probsT[:, tg * NH:(tg + 1) * NH],
                                         nats[q][:, tl * DC:(tl + 1) * DC],
                                         start=(tg == 0), stop=(tg == KT - 1))

                dsum = wpool.tile([NH, 1], F32, tag="dsum", bufs=2, name="dsum")
                nc.vector.tensor_reduce(dsum, den, axis=mybir.AxisListType.X,
                                        op=ALU.add)
                rin = wpool.tile([NH, 1], F32, tag="rin", bufs=2, name="rin")
                nc.vector.reciprocal(rin, dsum)
                attn_sb = cpool.tile([NH, DC], F32, tag=f"attn{b}",
                                     name=f"attn_sb{b}")
                nc.scalar.activation(attn_sb, attn_ps, AF.Copy, scale=rin)
                attn_sbs.append(attn_sb)

        # ---------------- stage C: W_UV absorption + output projection ----------------
        with tc.tile_pool(name="psC", bufs=1, space="PSUM") as psC:
            attnT = cpool.tile([128, 4 * NH * BPC], BF16)   # [c%128, (j, h, b)]
            av = attnT.rearrange("p (j h b) -> p j h b", j=4, h=NH, b=BPC)
            for b in range(BPC):
                for j in range(4):
                    ap_ = psC.tile([128, NH], F32, tag="att", bufs=2, name="ap_")
                    nc.tensor.transpose(ap_, attn_sbs[b][:, j * 128:(j + 1) * 128],
                                        id_f32[0:NH, 0:NH])
                    nc.vector.tensor_copy(av[:, j, :, b], ap_)

            vT = cpool.tile([128, NH * BPC], BF16)          # [dv, (h, b)]
            wuv_v = wuv_sb.rearrange("p (h j v) -> p h j v", h=NH, j=4, v=DV)
            for h in range(NH):
                vps = psC.tile([128, BPC], F32, tag="vt", bufs=2, name="vps")
                for j in range(4):
                    nc.tensor.matmul(vps, wuv_v[:, h, j, :], av[:, j, h, :],
                                     start=(j == 0), stop=(j == 3))
                nc.vector.tensor_copy(vT[:, h * BPC:(h + 1) * BPC], vps)

            y_ps = [psC.tile([BPC, 512], F32, tag="y", bufs=4, name=f"y{n}")
                    for n in range(4)]
            for h in range(NH):
                wo_t = wpool.tile([128, H], BF16, tag="wo", bufs=3, name="wo_t")
                nc.sync.dma_start(wo_t, t["wo"][h * DV:(h + 1) * DV, :])
                for n in range(4):
                    nc.tensor.matmul(y_ps[n], vT[:, h * BPC:(h + 1) * BPC],
                                     wo_t[:, n * 512:(n + 1) * 512],
                                     start=(h == 0), stop=(h == NH - 1))
            y_sb = cpool.tile([BPC, H], F32)
            for n in range(4):
                nc.scalar.copy(y_sb[:, n * 512:(n + 1) * 512], y_ps[n])
            nc.sync.dma_start(t["out"][:, :], y_sb)


def build_module(debug=False):
    nc = bacc.Bacc("TRN2", target_bir_lowering=False, debug=debug,
                   num_devices=N_CORES)
    t = {}
    t["ckv_nat"] = nc.dram_tensor("ckv_nat", [BPC, KVLEN, DC], BF16,
                                  kind="ExternalInput")
    t["ckv_t"] = nc.dram_tensor("ckv_t", [BPC, DC, KVLEN], BF16,
                                kind="ExternalInput")
    t["kpe_t"] = nc.dram_tensor("kpe_t", [BPC, DR, KVLEN], BF16,
                                kind="ExternalInput")
    t["hidT"] = nc.dram_tensor("hidT", [128, 16 * BSZ], BF16,
                               kind="ExternalInput")
    t["hidT_kva"] = nc.dram_tensor("hidT_kva", [128, 2 * BSZ], BF16,
                                   kind="ExternalInput")
    t["wuqr"] = nc.dram_tensor("wuqr", [H, 2 * DQ], BF16, kind="ExternalInput")
    t["wukt"] = nc.dram_tensor("wukt", [128, 2 * DC], BF16,
                               kind="ExternalInput")
    t["wkva"] = nc.dram_tensor("wkva", [2 * 128, DC + DR], BF16,
                               kind="ExternalInput")
    t["wuv"] = nc.dram_tensor("wuv", [128, NH * 4 * DV], BF16,
                              kind="ExternalInput")
    t["wo"] = nc.dram_tensor("wo", [NH * DV, H], BF16, kind="ExternalInput")
    t["lnw"] = nc.dram_tensor("lnw", [BPC, DC], F32, kind="ExternalInput")
    t["cosT"] = nc.dram_tensor("cosT", [DR, 1], F32, kind="ExternalInput")
    t["sinT"] = nc.dram_tensor("sinT", [DR, 1], F32, kind="ExternalInput")
    t["out"] = nc.dram_tensor("out", [BPC, H], F32, kind="ExternalOutput")

    with tile.TileContext(nc) as tc:
        _emit(tc, t)
    nc.compile()
    return nc


def prep_inputs(hidden_states, compressed_kv_normed_cache, k_pe_cache,
                W_UQR, W_kva, ln_w, W_UK, W_UV, W_O, cos, sin):
    """Host-side layout/dtype prep + per-core sharding. Returns in_maps."""
    bf16 = ml_dtypes.bfloat16
    f32 = np.float32

    # W_UK [h, c, d] -> [d, (h c)]
    wukt_full = np.ascontiguousarray(
        np.asarray(W_UK).transpose(2, 0, 1)).astype(bf16)       # [128, 16, 512]
    wuqr_h = np.asarray(W_UQR).reshape(H, NH, DQ)
    wkva_f = np.asarray(W_kva)
    # W_UV [h, c, v] -> [c%128, (h, j, v)]
    wuv = np.asarray(W_UV).reshape(NH, 4, 128, DV).transpose(2, 0, 1, 3)
    wuv = np.ascontiguousarray(wuv.reshape(128, NH * 4 * DV)).astype(bf16)
    wo = np.ascontiguousarray(np.asarray(W_O)).astype(bf16)
    lnw = np.tile(np.asarray(ln_w, dtype=f32)[None, :], (BPC, 1))
    cosT = np.ascontiguousarray(np.asarray(cos, dtype=f32).reshape(1, DR).T)
    sinT = np.ascontiguousarray(np.asarray(sin, dtype=f32).reshape(1, DR).T)

    ckv = np.asarray(compressed_kv_normed_cache)
    kpe = np.asarray(k_pe_cache)
    hs = np.asarray(hidden_states)

    ckv_nat = ckv.astype(bf16)                                   # [32, k, c]
    ckv_t = ckv.transpose(0, 2, 1).astype(bf16)                  # [32, c, k]
    ckv_t = np.ascontiguousarray(ckv_t)
    kpe_t = np.ascontiguousarray(kpe.transpose(0, 2, 1).astype(bf16))

    # hiddenT for all 32 sequences: [128, (i 16, B 32)]
    hidT3 = hs.T.reshape(16, 128, BSZ)
    hidT_full = np.ascontiguousarray(
        hidT3.transpose(1, 0, 2).reshape(128, 16 * BSZ)).astype(bf16)

    in_maps = []
    for c in range(N_CORES):
        sl = slice(c * BPC, (c + 1) * BPC)
        hid_kva = np.ascontiguousarray(
            hidT3[2 * c:2 * c + 2].transpose(1, 0, 2).reshape(128, 2 * BSZ)
        ).astype(bf16)
        wuqr_c = np.ascontiguousarray(
            wuqr_h[:, 2 * c:2 * c + 2, :].reshape(H, 2 * DQ)).astype(bf16)
        wukt_c = np.ascontiguousarray(
            wukt_full[:, 2 * c:2 * c + 2, :].reshape(128, 2 * DC))
        wkva_c = np.ascontiguousarray(
            wkva_f[256 * c:256 * (c + 1), :]).astype(bf16)
        in_maps.append({
            "ckv_nat": np.ascontiguousarray(ckv_nat[sl]),
            "ckv_t": np.ascontiguousarray(ckv_t[sl]),
            "kpe_t": np.ascontiguousarray(kpe_t[sl]),
            "hidT": hidT_full, "hidT_kva": hid_kva,
            "wuqr": wuqr_c, "wukt": wukt_c, "wkva": wkva_c, "wuv": wuv,
            "wo": wo,
            "lnw": lnw.astype(f32), "cosT": cosT.astype(f32),
            "sinT": sinT.astype(f32),
        })
    return in_maps


_MODULE = None


def _get_module():
    global _MODULE
    if _MODULE is None:
        _MODULE = build_module()
    return _MODULE


def kernel(**inputs):
    nc = _get_module()
    in_maps = prep_inputs(**inputs)
    res = run_bass_kernel_spmd(nc, in_maps, core_ids=list(range(N_CORES)))
    out = np.concatenate([r["out"] for r in res.results], axis=0)
    return np.ascontiguousarray(out.astype(np.float32))


# revision 33
# speedup vs baseline: 398.6216x; 1.2685x over previous
"""DeepSeek-V2-Lite matrix-absorbed MLA decode on 8 Trainium2 NeuronCores.

Sharding: attention is data-parallel over batch (4 sequences + their KV cache
slices per core). The query/latent projections are tensor-parallel: each core
computes its 2 heads (W_UQR/W_UK column shard) and a W_kva row-shard partial
for ALL 32 sequences, then an AllToAll (q) + ReduceScatter (latent) hand every
core all 16 heads for its own 4 sequences. W_UV/W_O stay replicated (the
output-side collectives would sit on the critical-path tail).

Host-side input prep casts the KV caches and weights to bf16 and ships the
compressed-KV cache in both natural [k, c] and transposed [c, k] layouts so
both attention matmuls stream through the PE with no on-device transposes of
large tensors. Attention is a single flash pass: softmax skips the max
subtraction (|scores*scale| <= ~4 for this problem family, exp stays finite in
fp32) and the denominator comes from the Exp activation's accum_out.
"""

import sys

import numpy as np
import ml_dtypes

for _p in ("/opt/trn_rl_repo",):
    if _p not in sys.path:
        sys.path.insert(0, _p)

import concourse.bass as bass  # noqa: E402
import concourse.mybir as mybir  # noqa: E402
import concourse.tile as tile  # noqa: E402
from concourse import bacc  # noqa: E402
from concourse.bass_utils import run_bass_kernel_spmd  # noqa: E402
from concourse.masks import make_identity  # noqa: E402

# Problem constants (hardcoded per harness contract).
H = 2048
NH = 16
DR = 64
DC = 512
DV = 128
DN = 128
DQ = 192
EPS = 1e-6
SCALE = DQ ** -0.5
BSZ, KVLEN = 32, 4096

N_CORES = 8
BPC = BSZ // N_CORES          # sequences per core
KT = KVLEN // 128             # 32 k-tiles of 128 positions
NQ = 4                        # score quarters (psum-sized chunks of k)
KQ = KVLEN // NQ              # 1024 score columns per quarter
TQ = KQ // 128                # 8 k-tiles per quarter

BF16 = mybir.dt.bfloat16
F32 = mybir.dt.float32
AF = mybir.ActivationFunctionType
ALU = mybir.AluOpType


def _emit(tc, t):
    nc = tc.nc

    with tc.tile_pool(name="cpool", bufs=1) as cpool, \
         tc.tile_pool(name="wpool", bufs=2) as wpool, \
         tc.tile_pool(name="cachepool", bufs=2) as cachepool:

        # ---------------- constants / persistent small tensors ----------------
        id_bf = cpool.tile([128, 128], BF16)
        make_identity(nc, id_bf)
        id_f32 = cpool.tile([128, 128], F32)
        make_identity(nc, id_f32)

        hidT_sb = cpool.tile([128, 16 * BSZ], BF16)     # all 32 sequences
        nc.sync.dma_start(hidT_sb, t["hidT"][:, :])
        hidkva_sb = cpool.tile([128, 2 * BSZ], BF16)    # hid chunks for W_kva slice
        nc.sync.dma_start(hidkva_sb, t["hidT_kva"][:, :])
        wukt_sb = cpool.tile([128, 2 * DC], BF16)       # this core's 2 heads
        nc.sync.dma_start(wukt_sb, t["wukt"][:, :])
        cosT_sb = cpool.tile([DR, 1], F32)
        nc.sync.dma_start(cosT_sb, t["cosT"][:, :])
        sinT_sb = cpool.tile([DR, 1], F32)
        nc.sync.dma_start(sinT_sb, t["sinT"][:, :])
        lnw_sb = cpool.tile([BPC, DC], F32)
        nc.sync.dma_start(lnw_sb, t["lnw"][:, :])

        qabsT = cpool.tile([128, N_CORES * 4 * BPC * 2], BF16)  # [p,(s,j,bl,hl)]
        qpeT_b16 = cpool.tile([DR, N_CORES * BPC * 2], BF16)    # [r,(s,bl,hl)]
        cn_b16 = cpool.tile([BPC, DC], BF16)            # c_norm rows (natural fixup)
        cnT = cpool.tile([128, 4 * BPC], BF16)          # c_norm cols [(j, b)]
        kpenT_b16 = cpool.tile([DR, BPC], BF16)         # roped new k_pe cols

        def rope_cols(x_f32, out_b16, pool, nm):
            # rope along the partition (r) axis of [64, n]; cos/sin per-partition
            n = x_f32.shape[-1]
            rot = pool.tile([DR, n], F32, tag=f"rot{nm}", name=f"rot{nm}")
            nc.scalar.mul(rot[0:DR // 2, :], x_f32[DR // 2:DR, :], -1.0)
            nc.scalar.copy(rot[DR // 2:DR, :], x_f32[0:DR // 2, :])
            t1 = pool.tile([DR, n], F32, tag=f"t1{nm}", name=f"t1{nm}")
            nc.vector.tensor_scalar_mul(t1, x_f32, cosT_sb)
            nc.vector.tensor_scalar_mul(rot, rot, sinT_sb)
            nc.vector.tensor_add(t1, t1, rot)
            nc.vector.tensor_copy(out_b16, t1)

        # ---------------- stage A: sharded projections + exchange ----------------
        RG = [list(range(N_CORES))]
        with tc.tile_pool(name="psA", bufs=1, space="PSUM") as psA, \
             tc.tile_pool(name="dpool", bufs=1, space="DRAM") as dpool:
            # q for this core's 2 heads, ALL 32 sequences
            wuqr_sb = cpool.tile([128, 16 * 2 * DQ], BF16)
            wuqr_v = t["wuqr"].rearrange("(g i p) n -> g p i n", g=4, p=128)
            for g4 in range(4):
                nc.sync.dma_start(
                    wuqr_sb.rearrange("p (g i n) -> g p i n", g=4, i=4)[g4],
                    wuqr_v[g4])
            q_ps = psA.tile([BSZ, 2 * DQ], F32, tag="qps", bufs=1)
            for i in range(16):
                nc.tensor.matmul(q_ps, hidT_sb[:, i * BSZ:(i + 1) * BSZ],
                                 wuqr_sb[:, i * 2 * DQ:(i + 1) * 2 * DQ],
                                 start=(i == 0), stop=(i == 15))
            q_sb = cpool.tile([BSZ, 2 * DQ], F32)
            nc.scalar.copy(q_sb, q_ps)


            # this core's 2 heads: transposes + W_UK absorption -> send layout
            qsend_sb = cpool.tile([128, N_CORES * 4 * BPC * 2], BF16)
            qpesend_sb = cpool.tile([DR, N_CORES * BPC * 2], BF16)
            qpe2_f32 = cpool.tile([DR, 2 * BSZ], F32)
            qs_v = qsend_sb.rearrange("p (d j bl hl) -> p d j bl hl",
                                      d=N_CORES, j=4, bl=BPC, hl=2)
            for hl in range(2):
                tpn = psA.tile([128, BSZ], F32, tag="small", bufs=2, name="tpn")
                nc.tensor.transpose(tpn, q_sb[:, hl * DQ:hl * DQ + DN],
                                    id_f32[0:BSZ, 0:BSZ])
                qnT = wpool.tile([128, BSZ], BF16, tag="qnT", bufs=2, name="qnT")
                nc.vector.tensor_copy(qnT, tpn)
                aps = psA.tile([BSZ, DC], F32, tag="small", bufs=2, name="aps")
                nc.tensor.matmul(aps, qnT, wukt_sb[:, hl * DC:(hl + 1) * DC],
                                 start=True, stop=True)
                qabs_sb = wpool.tile([BSZ, DC], F32, tag="qabs_sb", bufs=2,
                                     name="qabs_sb")
                nc.scalar.copy(qabs_sb, aps)
                for j in range(4):
                    tpa = psA.tile([128, BSZ], F32, tag="small", bufs=2, name="tpa")
                    nc.tensor.transpose(tpa, qabs_sb[:, j * 128:(j + 1) * 128],
                                        id_f32[0:BSZ, 0:BSZ])
                    nc.vector.tensor_copy(
                        qs_v[:, :, j, :, hl],
                        tpa.rearrange("p (d bl) -> p d bl", d=N_CORES))
                tpp = psA.tile([DR, BSZ], F32, tag="small", bufs=2, name="tpp")
                nc.tensor.transpose(tpp, q_sb[:, hl * DQ + DN:(hl + 1) * DQ],
                                    id_f32[0:BSZ, 0:BSZ])
                nc.vector.tensor_copy(qpe2_f32[:, hl * BSZ:(hl + 1) * BSZ], tpp)
            qpe2_roped = cpool.tile([DR, 2 * BSZ], F32)
            rope_cols(qpe2_f32, qpe2_roped, cpool, "q")
            qpv = qpesend_sb.rearrange("r (d bl hl) -> r d bl hl",
                                       d=N_CORES, bl=BPC, hl=2)
            for hl in range(2):
                nc.vector.tensor_copy(
                    qpv[:, :, :, hl],
                    qpe2_roped[:, hl * BSZ:(hl + 1) * BSZ].rearrange(
                        "r (d bl) -> r d bl", d=N_CORES))

            # AllToAll: each core ends with all 16 heads for its 4 sequences
            QCH = 4 * BPC * 2 * 128 + BPC * 2 * DR     # per-dest chunk (elems)
            QA = 4 * BPC * 2 * 128                     # qabs region size
            qsend_d = dpool.tile([N_CORES, QCH], BF16, name="qsend_d")
            nc.sync.dma_start(
                qsend_d[:, 0:QA].rearrange("d (p c) -> p d c", p=128),
                qsend_sb.rearrange("p (d c) -> p d c", d=N_CORES))
            nc.sync.dma_start(
                qsend_d[:, QA:QCH].rearrange("d (r c) -> r d c", r=DR),
                qpesend_sb.rearrange("r (d c) -> r d c", d=N_CORES))
            qrecv_d = dpool.tile([N_CORES, QCH], BF16, name="qrecv_d")
            nc.gpsimd.collective_compute("AllToAll", ALU.bypass, RG,
                                         [qsend_d[:, :]], [qrecv_d[:, :]])
            # land src-major (simple 3-dim DMA), then one DVE copy reorders so
            # the 16 head columns (src, hl) are contiguous per (j, bl) — the
            # scores lhsT slices must be plain 2-D APs for walrus
            qabs_raw = cpool.tile([128, N_CORES * 4 * BPC * 2], BF16)
            nc.sync.dma_start(
                qabs_raw.rearrange("p (s c) -> p s c", s=N_CORES),
                qrecv_d[:, 0:QA].rearrange("s (p c) -> p s c", p=128))
            qpe_raw = cpool.tile([DR, N_CORES * BPC * 2], BF16)
            nc.sync.dma_start(
                qpe_raw.rearrange("r (s c) -> r s c", s=N_CORES),
                qrecv_d[:, QA:QCH].rearrange("s (r c) -> r s c", r=DR))
            nc.vector.tensor_copy(
                qabsT.rearrange("p (j bl s hl) -> p s j bl hl",
                                j=4, bl=BPC, s=N_CORES),
                qabs_raw.rearrange("p (s j bl hl) -> p s j bl hl",
                                   s=N_CORES, j=4, bl=BPC))
            nc.vector.tensor_copy(
                qpeT_b16.rearrange("r (bl s hl) -> r s bl hl",
                                   bl=BPC, s=N_CORES),
                qpe_raw.rearrange("r (s bl hl) -> r s bl hl",
                                  s=N_CORES, bl=BPC))

            # partial latent from this core's W_kva row-slice, ReduceScatter(add)
            wkva_sb = cpool.tile([128, 2 * (DC + DR)], BF16)
            nc.sync.dma_start(wkva_sb.rearrange("p (c n) -> p c n", c=2),
                              t["wkva"].rearrange("(c p) n -> p c n", p=128))
            lat_ps = psA.tile([BSZ, DC + DR], F32, tag="latps", bufs=1)
            for c in range(2):
                lhsT = hidkva_sb[:, c * BSZ:(c + 1) * BSZ]
                w0 = c * (DC + DR)
                nc.tensor.matmul(lat_ps[:, 0:DC], lhsT, wkva_sb[:, w0:w0 + DC],
                                 start=(c == 0), stop=(c == 1))
                nc.tensor.matmul(lat_ps[:, DC:DC + DR], lhsT,
                                 wkva_sb[:, w0 + DC:w0 + DC + DR],
                                 start=(c == 0), stop=(c == 1))
            latp_sb = wpool.tile([BSZ, DC + DR], F32, tag="latp", name="latp_sb")
            nc.scalar.copy(latp_sb, lat_ps)
            latp_d = dpool.tile([BSZ, DC + DR], F32, name="latp_d")
            nc.sync.dma_start(latp_d, latp_sb)
            latr_d = dpool.tile([BPC, DC + DR], F32, name="latr_d")
            nc.gpsimd.collective_compute("ReduceScatter", ALU.add, RG,
                                         [latp_d[:, :]], [latr_d[:, :]])
            lat_sb = cpool.tile([BPC, DC + DR], F32)
            nc.scalar.dma_start(lat_sb, latr_d[:, :])

            # rms_norm(latent[:, :512]) * ln_w
            sq = cpool.tile([BPC, DC], F32)
            ssq = cpool.tile([BPC, 1], F32)
            nc.scalar.activation(sq, lat_sb[:, :DC], AF.Square, accum_out=ssq)
            eps_sb = cpool.tile([BPC, 1], F32)
            nc.vector.memset(eps_sb, EPS)
            stdv = cpool.tile([BPC, 1], F32)
            nc.scalar.activation(stdv, ssq, AF.Sqrt, scale=1.0 / DC, bias=eps_sb)
            rinv = cpool.tile([BPC, 1], F32)
            nc.vector.reciprocal(rinv, stdv)
            cn = cpool.tile([BPC, DC], F32)
            nc.vector.tensor_scalar_mul(cn, lat_sb[:, :DC], rinv)
            nc.vector.tensor_mul(cn, cn, lnw_sb)
            nc.vector.tensor_copy(cn_b16, cn)
            for j in range(4):
                tp = psA.tile([128, BPC], F32, tag="small", bufs=2, name="tp")
                nc.tensor.transpose(tp, cn[:, j * 128:(j + 1) * 128],
                                    id_f32[0:BPC, 0:BPC])
                nc.vector.tensor_copy(cnT[:, j * BPC:(j + 1) * BPC], tp)

            # new-token k_pe: transpose then rope (cols)
            kpT = psA.tile([DR, BPC], F32, tag="small", bufs=2, name="kpT")
            nc.tensor.transpose(kpT, lat_sb[:, DC:DC + DR], id_f32[0:BPC, 0:BPC])
            kpe_f32 = cpool.tile([DR, BPC], F32)
            nc.vector.tensor_copy(kpe_f32, kpT)
            rope_cols(kpe_f32, kpenT_b16, cpool, "k")
        qa = qabsT.rearrange("p (j bl shl) -> p j bl shl", j=4, bl=BPC)
        qp = qpeT_b16.rearrange("r (bl shl) -> r bl shl", bl=BPC)

        wuv_sb = cpool.tile([128, NH * 4 * DV], BF16)
        nc.sync.dma_start(wuv_sb, t["wuv"][:, :])
        # W_O prefetch — emitted before the attention loop so the stream
        # overlaps the cache DMAs instead of sitting on the serial tail
        wo_tiles = []
        for h in range(NH):
            wo_t = wpool.tile([128, H], BF16, tag="wo", bufs=16, name="wo_t")
            nc.sync.dma_start(wo_t, t["wo"][h * DV:(h + 1) * DV, :])
            wo_tiles.append(wo_t)

        # ---------------- stage B: flash attention per sequence ----------------
        attn_sbs = []
        with tc.tile_pool(name="psB", bufs=1, space="PSUM") as psB:
            for b in range(BPC):
                natv = t["ckv_nat"][b].rearrange("(g t p) c -> g t p c",
                                                 p=128, t=TQ)
                # ckv_t [512, 4096] viewed [p(c%128), j, k] for packed loads
                ckvTj = t["ckv_t"][b].rearrange("(j p) k -> p j k", p=128)
                kpeTv = t["kpe_t"][b]

                # kpe first (rope term of every quarter's scores needs it),
                # then per-quarter [ckvT, nat] pairs so the ring FIFO delivers
                # each quarter's scores operand before its attnV operand
                kt_ = cachepool.tile([DR, KVLEN], BF16, tag="kpeT", bufs=2,
                                     name="kt_")
                nc.scalar.dma_start(kt_, kpeTv[:, :])
                nc.vector.tensor_copy(kt_[:, KVLEN - 1:KVLEN],
                                      kpenT_b16[:, b:b + 1])

                probs = cachepool.tile([NH, KVLEN], BF16, tag="probs", bufs=2,
                                       name="probs")
                probsT = cachepool.tile([128, KT * NH], BF16, tag="probsT", bufs=2,
                                        name="probsT")
                den = wpool.tile([NH, NQ], F32, tag="den", bufs=2, name="den")
                attn_ps = psB.tile([NH, DC], F32, tag="attn", bufs=2, name="attn_ps")

                nats = []
                for q in range(NQ):
                    ct = cachepool.tile([128, 4 * KQ], BF16, tag="ckvT", bufs=2,
                                        name="ct")
                    ctv = ct.rearrange("p (j k) -> p j k", j=4)
                    nc.scalar.dma_start(ctv, ckvTj[:, :, q * KQ:(q + 1) * KQ])
                    nat = cachepool.tile([128, TQ * DC], BF16, tag="nat", bufs=2,
                                         name="nat")
                    nc.scalar.dma_start(nat.rearrange("p (t c) -> p t c", t=TQ),
                                        natv[q].rearrange("t p c -> p t c"))
                    nats.append(nat)
                    if q == NQ - 1:
                        for j in range(4):
                            nc.vector.tensor_copy(
                                ctv[:, j, KQ - 1:KQ],
                                cnT[:, j * BPC + b:j * BPC + b + 1])
                        # normed new-token latent into the last cache slot (row
                        # 127 of the last k-tile) — DMA for cross-partition move
                        nc.scalar.dma_start(nat[127:128, (TQ - 1) * DC:TQ * DC],
                                            cn_b16[b:b + 1, :])

                    sc = psB.tile([NH, KQ], F32, tag="scores", bufs=2, name="sc")
                    for half in range(2):
                        csl = slice(half * 512, (half + 1) * 512)
                        for j in range(4):
                            nc.tensor.matmul(sc[:, csl], qa[:, j, b, :],
                                             ctv[:, j, csl],
                                             start=(j == 0), stop=False)
                        nc.tensor.matmul(sc[:, csl], qp[:, b, :],
                                         kt_[:, q * KQ:(q + 1) * KQ][:, csl],
                                         start=False, stop=True)
                    # exp (softmax numerator) + running denominator
                    nc.scalar.activation(probs[:, q * KQ:(q + 1) * KQ], sc, AF.Exp,
                                         scale=SCALE, accum_out=den[:, q:q + 1])
                    pT = psB.tile([128, TQ * NH], BF16, tag="pT", bufs=2, name="pT")
                    for tl in range(TQ):
                        nc.tensor.transpose(
                            pT[:, tl * NH:(tl + 1) * NH],
                            probs[:, q * KQ + tl * 128:q * KQ + (tl + 1) * 128],
                            id_bf[0:NH, 0:NH])
                    nc.vector.tensor_copy(
                        probsT[:, q * TQ * NH:(q + 1) * TQ * NH], pT)
                    for tl in range(TQ):
                        tg = q * TQ + tl
                        nc.tensor.matmul(attn_ps,
                                         probsT[:, tg * NH:(tg + 1) * NH],
                                         nats[q][:, tl * DC:(tl + 1) * DC],
                                         start=(tg == 0), stop=(tg == KT - 1))

                dsum = wpool.tile([NH, 1], F32, tag="dsum", bufs=2, name="dsum")
                nc.vector.tensor_reduce(dsum, den, axis=mybir.AxisListType.X,
                                        op=ALU.add)
                rin = wpool.tile([NH, 1], F32, tag="rin", bufs=2, name="rin")
                nc.vector.reciprocal(rin, dsum)
                attn_sb = cpool.tile([NH, DC], F32, tag=f"attn{b}",
                                     name=f"attn_sb{b}")
                nc.scalar.activation(attn_sb, attn_ps, AF.Copy, scale=rin)
                attn_sbs.append(attn_sb)

        # ---------------- stage C: W_UV absorption + output projection ----------------
        with tc.tile_pool(name="psC", bufs=1, space="PSUM") as psC:
            attnT = cpool.tile([128, 4 * NH * BPC], BF16)   # [c%128, (j, h, b)]
            av = attnT.rearrange("p (j h b) -> p j h b", j=4, h=NH, b=BPC)
            for b in range(BPC):
                for j in range(4):
                    ap_ = psC.tile([128, NH], F32, tag="att", bufs=2, name="ap_")
                    nc.tensor.transpose(ap_, attn_sbs[b][:, j * 128:(j + 1) * 128],
                                        id_f32[0:NH, 0:NH])
                    nc.vector.tensor_copy(av[:, j, :, b], ap_)

            vT = cpool.tile([128, NH * BPC], BF16)          # [dv, (h, b)]
            wuv_v = wuv_sb.rearrange("p (h j v) -> p h j v", h=NH, j=4, v=DV)
            for h in range(NH):
                vps = psC.tile([128, BPC], F32, tag="vt", bufs=2, name="vps")
                for j in range(4):
                    nc.tensor.matmul(vps, wuv_v[:, h, j, :], av[:, j, h, :],
                                     start=(j == 0), stop=(j == 3))
                nc.vector.tensor_copy(vT[:, h * BPC:(h + 1) * BPC], vps)

            y_ps = [psC.tile([BPC, 512], F32, tag="y", bufs=4, name=f"y{n}")
                    for n in range(4)]
            for h in range(NH):
                for n in range(4):
                    nc.tensor.matmul(y_ps[n], vT[:, h * BPC:(h + 1) * BPC],
                                     wo_tiles[h][:, n * 512:(n + 1) * 512],
                                     start=(h == 0), stop=(h == NH - 1))
            y_sb = cpool.tile([BPC, H], F32)
            for n in range(4):
                nc.scalar.copy(y_sb[:, n * 512:(n + 1) * 512], y_ps[n])
            nc.sync.dma_start(t["out"][:, :], y_sb)


def build_module(debug=False):
    nc = bacc.Bacc("TRN2", target_bir_lowering=False, debug=debug,
                   num_devices=N_CORES)
    t = {}
    t["ckv_nat"] = nc.dram_tensor("ckv_nat", [BPC, KVLEN, DC], BF16,
                                  kind="ExternalInput")
    t["ckv_t"] = nc.dram_tensor("ckv_t", [BPC, DC, KVLEN], BF16,
                                kind="ExternalInput")
    t["kpe_t"] = nc.dram_tensor("kpe_t", [BPC, DR, KVLEN], BF16,
                                kind="ExternalInput")
    t["hidT"] = nc.dram_tensor("hidT", [128, 16 * BSZ], BF16,
                               kind="ExternalInput")
    t["hidT_kva"] = nc.dram_tensor("hidT_kva", [128, 2 * BSZ], BF16,
                                   kind="ExternalInput")
    t["wuqr"] = nc.dram_tensor("wuqr", [H, 2 * DQ], BF16, kind="ExternalInput")
    t["wukt"] = nc.dram_tensor("wukt", [128, 2 * DC], BF16,
                               kind="ExternalInput")
    t["wkva"] = nc.dram_tensor("wkva", [2 * 128, DC + DR], BF16,
                               kind="ExternalInput")
    t["wuv"] = nc.dram_tensor("wuv", [128, NH * 4 * DV], BF16,
                              kind="ExternalInput")
    t["wo"] = nc.dram_tensor("wo", [NH * DV, H], BF16, kind="ExternalInput")
    t["lnw"] = nc.dram_tensor("lnw", [BPC, DC], F32, kind="ExternalInput")
    t["cosT"] = nc.dram_tensor("cosT", [DR, 1], F32, kind="ExternalInput")
    t["sinT"] = nc.dram_tensor("sinT", [DR, 1], F32, kind="ExternalInput")
    t["out"] = nc.dram_tensor("out", [BPC, H], F32, kind="ExternalOutput")

    with tile.TileContext(nc) as tc:
        _emit(tc, t)
    nc.compile()
    return nc


def prep_inputs(hidden_states, compressed_kv_normed_cache, k_pe_cache,
                W_UQR, W_kva, ln_w, W_UK, W_UV, W_O, cos, sin):
    """Host-side layout/dtype prep + per-core sharding. Returns in_maps."""
    bf16 = ml_dtypes.bfloat16
    f32 = np.float32

    # W_UK [h, c, d] -> [d, (h c)]
    wukt_full = np.ascontiguousarray(
        np.asarray(W_UK).transpose(2, 0, 1)).astype(bf16)       # [128, 16, 512]
    wuqr_h = np.asarray(W_UQR).reshape(H, NH, DQ)
    wkva_f = np.asarray(W_kva)
    # W_UV [h, c, v] -> [c%128, (h, j, v)]
    wuv = np.asarray(W_UV).reshape(NH, 4, 128, DV).transpose(2, 0, 1, 3)
    wuv = np.ascontiguousarray(wuv.reshape(128, NH * 4 * DV)).astype(bf16)
    wo = np.ascontiguousarray(np.asarray(W_O)).astype(bf16)
    lnw = np.tile(np.asarray(ln_w, dtype=f32)[None, :], (BPC, 1))
    cosT = np.ascontiguousarray(np.asarray(cos, dtype=f32).reshape(1, DR).T)
    sinT = np.ascontiguousarray(np.asarray(sin, dtype=f32).reshape(1, DR).T)

    ckv = np.asarray(compressed_kv_normed_cache)
    kpe = np.asarray(k_pe_cache)
    hs = np.asarray(hidden_states)

    ckv_nat = ckv.astype(bf16)                                   # [32, k, c]
    ckv_t = ckv.transpose(0, 2, 1).astype(bf16)                  # [32, c, k]
    ckv_t = np.ascontiguousarray(ckv_t)
    kpe_t = np.ascontiguousarray(kpe.transpose(0, 2, 1).astype(bf16))

    # hiddenT for all 32 sequences: [128, (i 16, B 32)]
    hidT3 = hs.T.reshape(16, 128, BSZ)
    hidT_full = np.ascontiguousarray(
        hidT3.transpose(1, 0, 2).reshape(128, 16 * BSZ)).astype(bf16)

    in_maps = []
    for c in range(N_CORES):
        sl = slice(c * BPC, (c + 1) * BPC)
        hid_kva = np.ascontiguousarray(
            hidT3[2 * c:2 * c + 2].transpose(1, 0, 2).reshape(128, 2 * BSZ)
        ).astype(bf16)
        wuqr_c = np.ascontiguousarray(
            wuqr_h[:, 2 * c:2 * c + 2, :].reshape(H, 2 * DQ)).astype(bf16)
        wukt_c = np.ascontiguousarray(
            wukt_full[:, 2 * c:2 * c + 2, :].reshape(128, 2 * DC))
        wkva_c = np.ascontiguousarray(
            wkva_f[256 * c:256 * (c + 1), :]).astype(bf16)
        in_maps.append({
            "ckv_nat": np.ascontiguousarray(ckv_nat[sl]),
            "ckv_t": np.ascontiguousarray(ckv_t[sl]),
            "kpe_t": np.ascontiguousarray(kpe_t[sl]),
            "hidT": hidT_full, "hidT_kva": hid_kva,
            "wuqr": wuqr_c, "wukt": wukt_c, "wkva": wkva_c, "wuv": wuv,
            "wo": wo,
            "lnw": lnw.astype(f32), "cosT": cosT.astype(f32),
            "sinT": sinT.astype(f32),
        })
    return in_maps


_MODULE = None


def _get_module():
    global _MODULE
    if _MODULE is None:
        _MODULE = build_module()
    return _MODULE


def kernel(**inputs):
    nc = _get_module()
    in_maps = prep_inputs(**inputs)
    res = run_bass_kernel_spmd(nc, in_maps, core_ids=list(range(N_CORES)))
    out = np.concatenate([r["out"] for r in res.results], axis=0)
    return np.ascontiguousarray(out.astype(np.float32))


# revision 35
# speedup vs baseline: 403.6692x; 1.0127x over previous
"""DeepSeek-V2-Lite matrix-absorbed MLA decode on 8 Trainium2 NeuronCores.

Sharding: attention is data-parallel over batch (4 sequences + their KV cache
slices per core). The query/latent projections are tensor-parallel: each core
computes its 2 heads (W_UQR/W_UK column shard) and a W_kva row-shard partial
for ALL 32 sequences, then an AllToAll (q) + ReduceScatter (latent) hand every
core all 16 heads for its own 4 sequences. W_UV/W_O stay replicated (the
output-side collectives would sit on the critical-path tail).

Host-side input prep casts the KV caches and weights to bf16 and ships the
compressed-KV cache in both natural [k, c] and transposed [c, k] layouts so
both attention matmuls stream through the PE with no on-device transposes of
large tensors. Attention is a single flash pass: softmax skips the max
subtraction (|scores*scale| <= ~4 for this problem family, exp stays finite in
fp32) and the denominator comes from the Exp activation's accum_out.
"""

import sys

import numpy as np
import ml_dtypes

for _p in ("/opt/trn_rl_repo",):
    if _p not in sys.path:
        sys.path.insert(0, _p)

import concourse.bass as bass  # noqa: E402
import concourse.mybir as mybir  # noqa: E402
import concourse.tile as tile  # noqa: E402
from concourse import bacc  # noqa: E402
from concourse.bass_utils import run_bass_kernel_spmd  # noqa: E402
from concourse.masks import make_identity  # noqa: E402

# Problem constants (hardcoded per harness contract).
H = 2048
NH = 16
DR = 64
DC = 512
DV = 128
DN = 128
DQ = 192
EPS = 1e-6
SCALE = DQ ** -0.5
BSZ, KVLEN = 32, 4096

N_CORES = 8
BPC = BSZ // N_CORES          # sequences per core
KT = KVLEN // 128             # 32 k-tiles of 128 positions
NQ = 4                        # score quarters (psum-sized chunks of k)
KQ = KVLEN // NQ              # 1024 score columns per quarter
TQ = KQ // 128                # 8 k-tiles per quarter

BF16 = mybir.dt.bfloat16
F32 = mybir.dt.float32
AF = mybir.ActivationFunctionType
ALU = mybir.AluOpType


def _emit(tc, t):
    nc = tc.nc

    with tc.tile_pool(name="cpool", bufs=1) as cpool, \
         tc.tile_pool(name="wpool", bufs=2) as wpool:

        # ---------------- constants / persistent small tensors ----------------
        id_bf = cpool.tile([128, 128], BF16)
        make_identity(nc, id_bf)
        id_f32 = cpool.tile([128, 128], F32)
        make_identity(nc, id_f32)

        cosT_sb = cpool.tile([DR, 1], F32)
        nc.sync.dma_start(cosT_sb, t["cosT"][:, :])
        sinT_sb = cpool.tile([DR, 1], F32)
        nc.sync.dma_start(sinT_sb, t["sinT"][:, :])
        lnw_sb = cpool.tile([BPC, DC], F32)
        nc.sync.dma_start(lnw_sb, t["lnw"][:, :])

        qabsT = cpool.tile([128, N_CORES * 4 * BPC * 2], BF16)  # [p,(s,j,bl,hl)]
        qpeT_b16 = cpool.tile([DR, N_CORES * BPC * 2], BF16)    # [r,(s,bl,hl)]
        cn_b16 = cpool.tile([BPC, DC], BF16)            # c_norm rows (natural fixup)
        cnT = cpool.tile([128, 4 * BPC], BF16)          # c_norm cols [(j, b)]
        kpenT_b16 = cpool.tile([DR, BPC], BF16)         # roped new k_pe cols

        def rope_cols(x_f32, out_b16, pool, nm):
            # rope along the partition (r) axis of [64, n]; cos/sin per-partition
            n = x_f32.shape[-1]
            rot = pool.tile([DR, n], F32, tag=f"rot{nm}", name=f"rot{nm}")
            nc.scalar.mul(rot[0:DR // 2, :], x_f32[DR // 2:DR, :], -1.0)
            nc.scalar.copy(rot[DR // 2:DR, :], x_f32[0:DR // 2, :])
            t1 = pool.tile([DR, n], F32, tag=f"t1{nm}", name=f"t1{nm}")
            nc.vector.tensor_scalar_mul(t1, x_f32, cosT_sb)
            nc.vector.tensor_scalar_mul(rot, rot, sinT_sb)
            nc.vector.tensor_add(t1, t1, rot)
            nc.vector.tensor_copy(out_b16, t1)

        # ---------------- stage A: sharded projections + exchange ----------------
        RG = [list(range(N_CORES))]
        with tc.tile_pool(name="psA", bufs=1, space="PSUM") as psA, \
             tc.tile_pool(name="apool", bufs=1) as apool, \
             tc.tile_pool(name="dpool", bufs=1, space="DRAM") as dpool:
            hidT_sb = apool.tile([128, 16 * BSZ], BF16)
            nc.sync.dma_start(hidT_sb, t["hidT"][:, :])
            hidkva_sb = apool.tile([128, 2 * BSZ], BF16)
            nc.sync.dma_start(hidkva_sb, t["hidT_kva"][:, :])
            wukt_sb = apool.tile([128, 2 * DC], BF16)
            nc.sync.dma_start(wukt_sb, t["wukt"][:, :])
            # q for this core's 2 heads, ALL 32 sequences
            wuqr_sb = apool.tile([128, 16 * 2 * DQ], BF16)
            wuqr_v = t["wuqr"].rearrange("(g i p) n -> g p i n", g=4, p=128)
            for g4 in range(4):
                nc.sync.dma_start(
                    wuqr_sb.rearrange("p (g i n) -> g p i n", g=4, i=4)[g4],
                    wuqr_v[g4])
            q_ps = psA.tile([BSZ, 2 * DQ], F32, tag="qps", bufs=1)
            for i in range(16):
                nc.tensor.matmul(q_ps, hidT_sb[:, i * BSZ:(i + 1) * BSZ],
                                 wuqr_sb[:, i * 2 * DQ:(i + 1) * 2 * DQ],
                                 start=(i == 0), stop=(i == 15))
            q_sb = apool.tile([BSZ, 2 * DQ], F32)
            nc.scalar.copy(q_sb, q_ps)


            # this core's 2 heads: transposes + W_UK absorption -> send layout
            qsend_sb = apool.tile([128, N_CORES * 4 * BPC * 2], BF16)
            qpesend_sb = apool.tile([DR, N_CORES * BPC * 2], BF16)
            qpe2_f32 = apool.tile([DR, 2 * BSZ], F32)
            qs_v = qsend_sb.rearrange("p (d j bl hl) -> p d j bl hl",
                                      d=N_CORES, j=4, bl=BPC, hl=2)
            for hl in range(2):
                tpn = psA.tile([128, BSZ], F32, tag="small", bufs=2, name="tpn")
                nc.tensor.transpose(tpn, q_sb[:, hl * DQ:hl * DQ + DN],
                                    id_f32[0:BSZ, 0:BSZ])
                qnT = wpool.tile([128, BSZ], BF16, tag="qnT", bufs=2, name="qnT")
                nc.vector.tensor_copy(qnT, tpn)
                aps = psA.tile([BSZ, DC], F32, tag="small", bufs=2, name="aps")
                nc.tensor.matmul(aps, qnT, wukt_sb[:, hl * DC:(hl + 1) * DC],
                                 start=True, stop=True)
                qabs_sb = wpool.tile([BSZ, DC], F32, tag="qabs_sb", bufs=2,
                                     name="qabs_sb")
                nc.scalar.copy(qabs_sb, aps)
                for j in range(4):
                    tpa = psA.tile([128, BSZ], F32, tag="small", bufs=2, name="tpa")
                    nc.tensor.transpose(tpa, qabs_sb[:, j * 128:(j + 1) * 128],
                                        id_f32[0:BSZ, 0:BSZ])
                    nc.vector.tensor_copy(
                        qs_v[:, :, j, :, hl],
                        tpa.rearrange("p (d bl) -> p d bl", d=N_CORES))
                tpp = psA.tile([DR, BSZ], F32, tag="small", bufs=2, name="tpp")
                nc.tensor.transpose(tpp, q_sb[:, hl * DQ + DN:(hl + 1) * DQ],
                                    id_f32[0:BSZ, 0:BSZ])
                nc.vector.tensor_copy(qpe2_f32[:, hl * BSZ:(hl + 1) * BSZ], tpp)
            qpe2_roped = apool.tile([DR, 2 * BSZ], F32)
            rope_cols(qpe2_f32, qpe2_roped, apool, "q")
            qpv = qpesend_sb.rearrange("r (d bl hl) -> r d bl hl",
                                       d=N_CORES, bl=BPC, hl=2)
            for hl in range(2):
                nc.vector.tensor_copy(
                    qpv[:, :, :, hl],
                    qpe2_roped[:, hl * BSZ:(hl + 1) * BSZ].rearrange(
                        "r (d bl) -> r d bl", d=N_CORES))

            # AllToAll: each core ends with all 16 heads for its 4 sequences
            QCH = 4 * BPC * 2 * 128 + BPC * 2 * DR     # per-dest chunk (elems)
            QA = 4 * BPC * 2 * 128                     # qabs region size
            qsend_d = dpool.tile([N_CORES, QCH], BF16, name="qsend_d")
            nc.sync.dma_start(
                qsend_d[:, 0:QA].rearrange("d (p c) -> p d c", p=128),
                qsend_sb.rearrange("p (d c) -> p d c", d=N_CORES))
            nc.sync.dma_start(
                qsend_d[:, QA:QCH].rearrange("d (r c) -> r d c", r=DR),
                qpesend_sb.rearrange("r (d c) -> r d c", d=N_CORES))
            qrecv_d = dpool.tile([N_CORES, QCH], BF16, name="qrecv_d")
            nc.gpsimd.collective_compute("AllToAll", ALU.bypass, RG,
                                         [qsend_d[:, :]], [qrecv_d[:, :]])
            # land src-major (simple 3-dim DMA), then one DVE copy reorders so
            # the 16 head columns (src, hl) are contiguous per (j, bl) — the
            # scores lhsT slices must be plain 2-D APs for walrus
            qabs_raw = apool.tile([128, N_CORES * 4 * BPC * 2], BF16)
            nc.sync.dma_start(
                qabs_raw.rearrange("p (s c) -> p s c", s=N_CORES),
                qrecv_d[:, 0:QA].rearrange("s (p c) -> p s c", p=128))
            qpe_raw = apool.tile([DR, N_CORES * BPC * 2], BF16)
            nc.sync.dma_start(
                qpe_raw.rearrange("r (s c) -> r s c", s=N_CORES),
                qrecv_d[:, QA:QCH].rearrange("s (r c) -> r s c", r=DR))
            nc.vector.tensor_copy(
                qabsT.rearrange("p (j bl s hl) -> p s j bl hl",
                                j=4, bl=BPC, s=N_CORES),
                qabs_raw.rearrange("p (s j bl hl) -> p s j bl hl",
                                   s=N_CORES, j=4, bl=BPC))
            nc.vector.tensor_copy(
                qpeT_b16.rearrange("r (bl s hl) -> r s bl hl",
                                   bl=BPC, s=N_CORES),
                qpe_raw.rearrange("r (s bl hl) -> r s bl hl",
                                  s=N_CORES, bl=BPC))

            # partial latent from this core's W_kva row-slice, ReduceScatter(add)
            wkva_sb = apool.tile([128, 2 * (DC + DR)], BF16)
            nc.sync.dma_start(wkva_sb.rearrange("p (c n) -> p c n", c=2),
                              t["wkva"].rearrange("(c p) n -> p c n", p=128))
            lat_ps = psA.tile([BSZ, DC + DR], F32, tag="latps", bufs=1)
            for c in range(2):
                lhsT = hidkva_sb[:, c * BSZ:(c + 1) * BSZ]
                w0 = c * (DC + DR)
                nc.tensor.matmul(lat_ps[:, 0:DC], lhsT, wkva_sb[:, w0:w0 + DC],
                                 start=(c == 0), stop=(c == 1))
                nc.tensor.matmul(lat_ps[:, DC:DC + DR], lhsT,
                                 wkva_sb[:, w0 + DC:w0 + DC + DR],
                                 start=(c == 0), stop=(c == 1))
            latp_sb = wpool.tile([BSZ, DC + DR], F32, tag="latp", name="latp_sb")
            nc.scalar.copy(latp_sb, lat_ps)
            latp_d = dpool.tile([BSZ, DC + DR], F32, name="latp_d")
            nc.sync.dma_start(latp_d, latp_sb)
            latr_d = dpool.tile([BPC, DC + DR], F32, name="latr_d")
            nc.gpsimd.collective_compute("ReduceScatter", ALU.add, RG,
                                         [latp_d[:, :]], [latr_d[:, :]])
            lat_sb = apool.tile([BPC, DC + DR], F32)
            nc.scalar.dma_start(lat_sb, latr_d[:, :])

            # rms_norm(latent[:, :512]) * ln_w
            sq = apool.tile([BPC, DC], F32)
            ssq = apool.tile([BPC, 1], F32)
            nc.scalar.activation(sq, lat_sb[:, :DC], AF.Square, accum_out=ssq)
            eps_sb = apool.tile([BPC, 1], F32)
            nc.vector.memset(eps_sb, EPS)
            stdv = apool.tile([BPC, 1], F32)
            nc.scalar.activation(stdv, ssq, AF.Sqrt, scale=1.0 / DC, bias=eps_sb)
            rinv = apool.tile([BPC, 1], F32)
            nc.vector.reciprocal(rinv, stdv)
            cn = apool.tile([BPC, DC], F32)
            nc.vector.tensor_scalar_mul(cn, lat_sb[:, :DC], rinv)
            nc.vector.tensor_mul(cn, cn, lnw_sb)
            nc.vector.tensor_copy(cn_b16, cn)
            for j in range(4):
                tp = psA.tile([128, BPC], F32, tag="small", bufs=2, name="tp")
                nc.tensor.transpose(tp, cn[:, j * 128:(j + 1) * 128],
                                    id_f32[0:BPC, 0:BPC])
                nc.vector.tensor_copy(cnT[:, j * BPC:(j + 1) * BPC], tp)

            # new-token k_pe: transpose then rope (cols)
            kpT = psA.tile([DR, BPC], F32, tag="small", bufs=2, name="kpT")
            nc.tensor.transpose(kpT, lat_sb[:, DC:DC + DR], id_f32[0:BPC, 0:BPC])
            kpe_f32 = apool.tile([DR, BPC], F32)
            nc.vector.tensor_copy(kpe_f32, kpT)
            rope_cols(kpe_f32, kpenT_b16, apool, "k")
        qa = qabsT.rearrange("p (j bl shl) -> p j bl shl", j=4, bl=BPC)
        qp = qpeT_b16.rearrange("r (bl shl) -> r bl shl", bl=BPC)

        wuv_sb = cpool.tile([128, NH * 4 * DV], BF16)
        nc.sync.dma_start(wuv_sb, t["wuv"][:, :])
        # W_O prefetch — emitted before the attention loop so the stream
        # overlaps the cache DMAs instead of sitting on the serial tail
        wo_tiles = []
        for h in range(NH):
            wo_t = wpool.tile([128, H], BF16, tag="wo", bufs=16, name="wo_t")
            nc.sync.dma_start(wo_t, t["wo"][h * DV:(h + 1) * DV, :])
            wo_tiles.append(wo_t)

        # ---------------- stage B: flash attention per sequence ----------------
        attn_sbs = []
        with tc.tile_pool(name="psB", bufs=1, space="PSUM") as psB, \
             tc.tile_pool(name="cachepool", bufs=2) as cachepool:
            for b in range(BPC):
                natv = t["ckv_nat"][b].rearrange("(g t p) c -> g t p c",
                                                 p=128, t=TQ)
                # ckv_t [512, 4096] viewed [p(c%128), j, k] for packed loads
                ckvTj = t["ckv_t"][b].rearrange("(j p) k -> p j k", p=128)
                kpeTv = t["kpe_t"][b]

                # kpe first (rope term of every quarter's scores needs it),
                # then per-quarter [ckvT, nat] pairs so the ring FIFO delivers
                # each quarter's scores operand before its attnV operand
                kt_ = cachepool.tile([DR, KVLEN], BF16, tag="kpeT", bufs=2,
                                     name="kt_")
                nc.scalar.dma_start(kt_, kpeTv[:, :])
                nc.vector.tensor_copy(kt_[:, KVLEN - 1:KVLEN],
                                      kpenT_b16[:, b:b + 1])

                probs = cachepool.tile([NH, KVLEN], BF16, tag="probs", bufs=2,
                                       name="probs")
                probsT = cachepool.tile([128, KT * NH], BF16, tag="probsT", bufs=2,
                                        name="probsT")
                den = wpool.tile([NH, NQ], F32, tag="den", bufs=2, name="den")
                attn_ps = psB.tile([NH, DC], F32, tag="attn", bufs=2, name="attn_ps")

                nats = []
                for q in range(NQ):
                    ct = cachepool.tile([128, 4 * KQ], BF16, tag="ckvT", bufs=3,
                                        name="ct")
                    ctv = ct.rearrange("p (j k) -> p j k", j=4)
                    nc.scalar.dma_start(ctv, ckvTj[:, :, q * KQ:(q + 1) * KQ])
                    nat = cachepool.tile([128, TQ * DC], BF16, tag="nat", bufs=3,
                                         name="nat")
                    nc.scalar.dma_start(nat.rearrange("p (t c) -> p t c", t=TQ),
                                        natv[q].rearrange("t p c -> p t c"))
                    nats.append(nat)
                    if q == NQ - 1:
                        for j in range(4):
                            nc.vector.tensor_copy(
                                ctv[:, j, KQ - 1:KQ],
                                cnT[:, j * BPC + b:j * BPC + b + 1])
                        # normed new-token latent into the last cache slot (row
                        # 127 of the last k-tile) — DMA for cross-partition move
                        nc.scalar.dma_start(nat[127:128, (TQ - 1) * DC:TQ * DC],
                                            cn_b16[b:b + 1, :])

                    sc = psB.tile([NH, KQ], F32, tag="scores", bufs=2, name="sc")
                    for half in range(2):
                        csl = slice(half * 512, (half + 1) * 512)
                        for j in range(4):
                            nc.tensor.matmul(sc[:, csl], qa[:, j, b, :],
                                             ctv[:, j, csl],
                                             start=(j == 0), stop=False)
                        nc.tensor.matmul(sc[:, csl], qp[:, b, :],
                                         kt_[:, q * KQ:(q + 1) * KQ][:, csl],
                                         start=False, stop=True)
                    # exp (softmax numerator) + running denominator
                    nc.scalar.activation(probs[:, q * KQ:(q + 1) * KQ], sc, AF.Exp,
                                         scale=SCALE, accum_out=den[:, q:q + 1])
                    pT = psB.tile([128, TQ * NH], BF16, tag="pT", bufs=2, name="pT")
                    for tl in range(TQ):
                        nc.tensor.transpose(
                            pT[:, tl * NH:(tl + 1) * NH],
                            probs[:, q * KQ + tl * 128:q * KQ + (tl + 1) * 128],
                            id_bf[0:NH, 0:NH])
                    nc.vector.tensor_copy(
                        probsT[:, q * TQ * NH:(q + 1) * TQ * NH], pT)
                    for tl in range(TQ):
                        tg = q * TQ + tl
                        nc.tensor.matmul(attn_ps,
                                         probsT[:, tg * NH:(tg + 1) * NH],
                                         nats[q][:, tl * DC:(tl + 1) * DC],
                                         start=(tg == 0), stop=(tg == KT - 1))

                dsum = wpool.tile([NH, 1], F32, tag="dsum", bufs=2, name="dsum")
                nc.vector.tensor_reduce(dsum, den, axis=mybir.AxisListType.X,
                                        op=ALU.add)
                rin = wpool.tile([NH, 1], F32, tag="rin", bufs=2, name="rin")
                nc.vector.reciprocal(rin, dsum)
                attn_sb = cpool.tile([NH, DC], F32, tag=f"attn{b}",
                                     name=f"attn_sb{b}")
                nc.scalar.activation(attn_sb, attn_ps, AF.Copy, scale=rin)
                attn_sbs.append(attn_sb)

        # ---------------- stage C: W_UV absorption + output projection ----------------
        with tc.tile_pool(name="psC", bufs=1, space="PSUM") as psC:
            attnT = cpool.tile([128, 4 * NH * BPC], BF16)   # [c%128, (j, h, b)]
            av = attnT.rearrange("p (j h b) -> p j h b", j=4, h=NH, b=BPC)
            for b in range(BPC):
                for j in range(4):
                    ap_ = psC.tile([128, NH], F32, tag="att", bufs=2, name="ap_")
                    nc.tensor.transpose(ap_, attn_sbs[b][:, j * 128:(j + 1) * 128],
                                        id_f32[0:NH, 0:NH])
                    nc.vector.tensor_copy(av[:, j, :, b], ap_)

            vT = cpool.tile([128, NH * BPC], BF16)          # [dv, (h, b)]
            wuv_v = wuv_sb.rearrange("p (h j v) -> p h j v", h=NH, j=4, v=DV)
            for h in range(NH):
                vps = psC.tile([128, BPC], F32, tag="vt", bufs=2, name="vps")
                for j in range(4):
                    nc.tensor.matmul(vps, wuv_v[:, h, j, :], av[:, j, h, :],
                                     start=(j == 0), stop=(j == 3))
                nc.vector.tensor_copy(vT[:, h * BPC:(h + 1) * BPC], vps)

            y_ps = [psC.tile([BPC, 512], F32, tag="y", bufs=4, name=f"y{n}")
                    for n in range(4)]
            for h in range(NH):
                for n in range(4):
                    nc.tensor.matmul(y_ps[n], vT[:, h * BPC:(h + 1) * BPC],
                                     wo_tiles[h][:, n * 512:(n + 1) * 512],
                                     start=(h == 0), stop=(h == NH - 1))
            y_sb = cpool.tile([BPC, H], F32)
            for n in range(4):
                nc.scalar.copy(y_sb[:, n * 512:(n + 1) * 512], y_ps[n])
            nc.sync.dma_start(t["out"][:, :], y_sb)


def build_module(debug=False):
    nc = bacc.Bacc("TRN2", target_bir_lowering=False, debug=debug,
                   num_devices=N_CORES)
    t = {}
    t["ckv_nat"] = nc.dram_tensor("ckv_nat", [BPC, KVLEN, DC], BF16,
                                  kind="ExternalInput")
    t["ckv_t"] = nc.dram_tensor("ckv_t", [BPC, DC, KVLEN], BF16,
                                kind="ExternalInput")
    t["kpe_t"] = nc.dram_tensor("kpe_t", [BPC, DR, KVLEN], BF16,
                                kind="ExternalInput")
    t["hidT"] = nc.dram_tensor("hidT", [128, 16 * BSZ], BF16,
                               kind="ExternalInput")
    t["hidT_kva"] = nc.dram_tensor("hidT_kva", [128, 2 * BSZ], BF16,
                                   kind="ExternalInput")
    t["wuqr"] = nc.dram_tensor("wuqr", [H, 2 * DQ], BF16, kind="ExternalInput")
    t["wukt"] = nc.dram_tensor("wukt", [128, 2 * DC], BF16,
                               kind="ExternalInput")
    t["wkva"] = nc.dram_tensor("wkva", [2 * 128, DC + DR], BF16,
                               kind="ExternalInput")
    t["wuv"] = nc.dram_tensor("wuv", [128, NH * 4 * DV], BF16,
                              kind="ExternalInput")
    t["wo"] = nc.dram_tensor("wo", [NH * DV, H], BF16, kind="ExternalInput")
    t["lnw"] = nc.dram_tensor("lnw", [BPC, DC], F32, kind="ExternalInput")
    t["cosT"] = nc.dram_tensor("cosT", [DR, 1], F32, kind="ExternalInput")
    t["sinT"] = nc.dram_tensor("sinT", [DR, 1], F32, kind="ExternalInput")
    t["out"] = nc.dram_tensor("out", [BPC, H], F32, kind="ExternalOutput")

    with tile.TileContext(nc) as tc:
        _emit(tc, t)
    nc.compile()
    return nc


def prep_inputs(hidden_states, compressed_kv_normed_cache, k_pe_cache,
                W_UQR, W_kva, ln_w, W_UK, W_UV, W_O, cos, sin):
    """Host-side layout/dtype prep + per-core sharding. Returns in_maps."""
    bf16 = ml_dtypes.bfloat16
    f32 = np.float32

    # W_UK [h, c, d] -> [d, (h c)]
    wukt_full = np.ascontiguousarray(
        np.asarray(W_UK).transpose(2, 0, 1)).astype(bf16)       # [128, 16, 512]
    wuqr_h = np.asarray(W_UQR).reshape(H, NH, DQ)
    wkva_f = np.asarray(W_kva)
    # W_UV [h, c, v] -> [c%128, (h, j, v)]
    wuv = np.asarray(W_UV).reshape(NH, 4, 128, DV).transpose(2, 0, 1, 3)
    wuv = np.ascontiguousarray(wuv.reshape(128, NH * 4 * DV)).astype(bf16)
    wo = np.ascontiguousarray(np.asarray(W_O)).astype(bf16)
    lnw = np.tile(np.asarray(ln_w, dtype=f32)[None, :], (BPC, 1))
    cosT = np.ascontiguousarray(np.asarray(cos, dtype=f32).reshape(1, DR).T)
    sinT = np.ascontiguousarray(np.asarray(sin, dtype=f32).reshape(1, DR).T)

    ckv = np.asarray(compressed_kv_normed_cache)
    kpe = np.asarray(k_pe_cache)
    hs = np.asarray(hidden_states)

    ckv_nat = ckv.astype(bf16)                                   # [32, k, c]
    ckv_t = ckv.transpose(0, 2, 1).astype(bf16)                  # [32, c, k]
    ckv_t = np.ascontiguousarray(ckv_t)
    kpe_t = np.ascontiguousarray(kpe.transpose(0, 2, 1).astype(bf16))

    # hiddenT for all 32 sequences: [128, (i 16, B 32)]
    hidT3 = hs.T.reshape(16, 128, BSZ)
    hidT_full = np.ascontiguousarray(
        hidT3.transpose(1, 0, 2).reshape(128, 16 * BSZ)).astype(bf16)

    in_maps = []
    for c in range(N_CORES):
        sl = slice(c * BPC, (c + 1) * BPC)
        hid_kva = np.ascontiguousarray(
            hidT3[2 * c:2 * c + 2].transpose(1, 0, 2).reshape(128, 2 * BSZ)
        ).astype(bf16)
        wuqr_c = np.ascontiguousarray(
            wuqr_h[:, 2 * c:2 * c + 2, :].reshape(H, 2 * DQ)).astype(bf16)
        wukt_c = np.ascontiguousarray(
            wukt_full[:, 2 * c:2 * c + 2, :].reshape(128, 2 * DC))
        wkva_c = np.ascontiguousarray(
            wkva_f[256 * c:256 * (c + 1), :]).astype(bf16)
        in_maps.append({
            "ckv_nat": np.ascontiguousarray(ckv_nat[sl]),
            "ckv_t": np.ascontiguousarray(ckv_t[sl]),
            "kpe_t": np.ascontiguousarray(kpe_t[sl]),
            "hidT": hidT_full, "hidT_kva": hid_kva,
            "wuqr": wuqr_c, "wukt": wukt_c, "wkva": wkva_c, "wuv": wuv,
            "wo": wo,
            "lnw": lnw.astype(f32), "cosT": cosT.astype(f32),
            "sinT": sinT.astype(f32),
        })
    return in_maps


_MODULE = None


def _get_module():
    global _MODULE
    if _MODULE is None:
        _MODULE = build_module()
    return _MODULE


def kernel(**inputs):
    nc = _get_module()
    in_maps = prep_inputs(**inputs)
    res = run_bass_kernel_spmd(nc, in_maps, core_ids=list(range(N_CORES)))
    out = np.concatenate([r["out"] for r in res.results], axis=0)
    return np.ascontiguousarray(out.astype(np.float32))


# revision 42
# speedup vs baseline: 408.9963x; 1.0132x over previous
"""DeepSeek-V2-Lite matrix-absorbed MLA decode on 8 Trainium2 NeuronCores.

Sharding: attention is data-parallel over batch (4 sequences + their KV cache
slices per core). The query projection is tensor-parallel: each core computes
its 2 heads (W_UQR/W_UK column shard) for ALL 32 sequences, then one AllToAll
hands every core all 16 heads for its own 4 sequences. The latent/W_kva
projection is computed locally per core for its own sequences (replicating the
small W_kva beats a ReduceScatter that would gate the cache fixups), and
W_UV/W_O stay replicated (output-side collectives would sit on the tail).

Host-side input prep casts the KV caches and weights to bf16 and ships the
compressed-KV cache in both natural [k, c] and transposed [c, k] layouts so
both attention matmuls stream through the PE with no on-device transposes of
large tensors. Attention is a single flash pass: softmax skips the max
subtraction (|scores*scale| <= ~4 for this problem family, exp stays finite in
fp32) and the denominator comes from the Exp activation's accum_out.
"""

import sys

import numpy as np
import ml_dtypes

for _p in ("/opt/trn_rl_repo",):
    if _p not in sys.path:
        sys.path.insert(0, _p)

import concourse.bass as bass  # noqa: E402
import concourse.mybir as mybir  # noqa: E402
import concourse.tile as tile  # noqa: E402
from concourse import bacc  # noqa: E402
from concourse.bass_utils import run_bass_kernel_spmd  # noqa: E402
from concourse.masks import make_identity  # noqa: E402

# Problem constants (hardcoded per harness contract).
H = 2048
NH = 16
DR = 64
DC = 512
DV = 128
DN = 128
DQ = 192
EPS = 1e-6
SCALE = DQ ** -0.5
BSZ, KVLEN = 32, 4096

N_CORES = 8
BPC = BSZ // N_CORES          # sequences per core
KT = KVLEN // 128             # 32 k-tiles of 128 positions
NQ = 4                        # score quarters (psum-sized chunks of k)
KQ = KVLEN // NQ              # 1024 score columns per quarter
TQ = KQ // 128                # 8 k-tiles per quarter

BF16 = mybir.dt.bfloat16
F32 = mybir.dt.float32
AF = mybir.ActivationFunctionType
ALU = mybir.AluOpType


def _emit(tc, t):
    nc = tc.nc

    with tc.tile_pool(name="cpool", bufs=1) as cpool, \
         tc.tile_pool(name="wpool", bufs=2) as wpool:

        # ---------------- constants / persistent small tensors ----------------
        id_bf = cpool.tile([128, 128], BF16)
        make_identity(nc, id_bf)
        id_f32 = cpool.tile([128, 128], F32)
        make_identity(nc, id_f32)

        cosT_sb = cpool.tile([DR, 1], F32)
        nc.sync.dma_start(cosT_sb, t["cosT"][:, :])
        sinT_sb = cpool.tile([DR, 1], F32)
        nc.sync.dma_start(sinT_sb, t["sinT"][:, :])
        lnw_sb = cpool.tile([BPC, DC], F32)
        nc.sync.dma_start(lnw_sb, t["lnw"][:, :])

        qabsT = cpool.tile([128, N_CORES * 4 * BPC * 2], BF16)  # [p,(s,j,bl,hl)]
        qpeT_b16 = cpool.tile([DR, N_CORES * BPC * 2], BF16)    # [r,(s,bl,hl)]
        cn_b16 = cpool.tile([BPC, DC], BF16)            # c_norm rows (natural fixup)
        cnT = cpool.tile([128, 4 * BPC], BF16)          # c_norm cols [(j, b)]
        kpenT_b16 = cpool.tile([DR, BPC], BF16)         # roped new k_pe cols

        def rope_cols(x_f32, out_b16, pool, nm):
            # rope along the partition (r) axis of [64, n]; cos/sin per-partition
            n = x_f32.shape[-1]
            rot = pool.tile([DR, n], F32, tag=f"rot{nm}", name=f"rot{nm}")
            nc.scalar.mul(rot[0:DR // 2, :], x_f32[DR // 2:DR, :], -1.0)
            nc.scalar.copy(rot[DR // 2:DR, :], x_f32[0:DR // 2, :])
            t1 = pool.tile([DR, n], F32, tag=f"t1{nm}", name=f"t1{nm}")
            nc.vector.tensor_scalar_mul(t1, x_f32, cosT_sb)
            nc.vector.tensor_scalar_mul(rot, rot, sinT_sb)
            nc.vector.tensor_add(t1, t1, rot)
            nc.vector.tensor_copy(out_b16, t1)

        # ---------------- stage A: sharded projections + exchange ----------------
        RG = [list(range(N_CORES))]
        with tc.tile_pool(name="psA", bufs=1, space="PSUM") as psA, \
             tc.tile_pool(name="apool", bufs=1) as apool, \
             tc.tile_pool(name="dpool", bufs=1, space="DRAM") as dpool:
            hidT_sb = apool.tile([128, 16 * BSZ], BF16)
            nc.sync.dma_start(hidT_sb, t["hidT"][:, :])
            hidkva_sb = apool.tile([128, 16 * BPC], BF16)
            nc.sync.dma_start(hidkva_sb, t["hidT_kva"][:, :])
            wukt_sb = apool.tile([128, 2 * DC], BF16)
            nc.sync.dma_start(wukt_sb, t["wukt"][:, :])
            # q for this core's 2 heads, ALL 32 sequences
            wuqr_sb = apool.tile([128, 16 * 2 * DQ], BF16)
            wuqr_v = t["wuqr"].rearrange("(g i p) n -> g p i n", g=4, p=128)
            for g4 in range(4):
                nc.sync.dma_start(
                    wuqr_sb.rearrange("p (g i n) -> g p i n", g=4, i=4)[g4],
                    wuqr_v[g4])
            q_ps = psA.tile([BSZ, 2 * DQ], F32, tag="qps", bufs=1)
            for i in range(16):
                nc.tensor.matmul(q_ps, hidT_sb[:, i * BSZ:(i + 1) * BSZ],
                                 wuqr_sb[:, i * 2 * DQ:(i + 1) * 2 * DQ],
                                 start=(i == 0), stop=(i == 15))
            q_sb = apool.tile([BSZ, 2 * DQ], F32)
            nc.scalar.copy(q_sb, q_ps)


            # this core's 2 heads: transposes + W_UK absorption -> send layout
            qsend_sb = apool.tile([128, N_CORES * 4 * BPC * 2], BF16)
            qpesend_sb = apool.tile([DR, N_CORES * BPC * 2], BF16)
            qpe2_f32 = apool.tile([DR, 2 * BSZ], F32)
            qs_v = qsend_sb.rearrange("p (d j bl hl) -> p d j bl hl",
                                      d=N_CORES, j=4, bl=BPC, hl=2)
            for hl in range(2):
                tpn = psA.tile([128, BSZ], F32, tag="small", bufs=2, name="tpn")
                nc.tensor.transpose(tpn, q_sb[:, hl * DQ:hl * DQ + DN],
                                    id_f32[0:BSZ, 0:BSZ])
                qnT = wpool.tile([128, BSZ], BF16, tag="qnT", bufs=2, name="qnT")
                nc.vector.tensor_copy(qnT, tpn)
                aps = psA.tile([BSZ, DC], F32, tag="small", bufs=2, name="aps")
                nc.tensor.matmul(aps, qnT, wukt_sb[:, hl * DC:(hl + 1) * DC],
                                 start=True, stop=True)
                qabs_sb = wpool.tile([BSZ, DC], F32, tag="qabs_sb", bufs=2,
                                     name="qabs_sb")
                nc.scalar.copy(qabs_sb, aps)
                for j in range(4):
                    tpa = psA.tile([128, BSZ], F32, tag="small", bufs=2, name="tpa")
                    nc.tensor.transpose(tpa, qabs_sb[:, j * 128:(j + 1) * 128],
                                        id_f32[0:BSZ, 0:BSZ])
                    nc.vector.tensor_copy(
                        qs_v[:, :, j, :, hl],
                        tpa.rearrange("p (d bl) -> p d bl", d=N_CORES))
                tpp = psA.tile([DR, BSZ], F32, tag="small", bufs=2, name="tpp")
                nc.tensor.transpose(tpp, q_sb[:, hl * DQ + DN:(hl + 1) * DQ],
                                    id_f32[0:BSZ, 0:BSZ])
                nc.vector.tensor_copy(qpe2_f32[:, hl * BSZ:(hl + 1) * BSZ], tpp)
            qpe2_roped = apool.tile([DR, 2 * BSZ], F32)
            rope_cols(qpe2_f32, qpe2_roped, apool, "q")
            qpv = qpesend_sb.rearrange("r (d bl hl) -> r d bl hl",
                                       d=N_CORES, bl=BPC, hl=2)
            for hl in range(2):
                nc.vector.tensor_copy(
                    qpv[:, :, :, hl],
                    qpe2_roped[:, hl * BSZ:(hl + 1) * BSZ].rearrange(
                        "r (d bl) -> r d bl", d=N_CORES))

            # AllToAll: each core ends with all 16 heads for its 4 sequences
            QCH = 4 * BPC * 2 * 128 + BPC * 2 * DR     # per-dest chunk (elems)
            QA = 4 * BPC * 2 * 128                     # qabs region size
            qsend_d = dpool.tile([N_CORES, QCH], BF16, name="qsend_d")
            nc.sync.dma_start(
                qsend_d[:, 0:QA].rearrange("d (p c) -> p d c", p=128),
                qsend_sb.rearrange("p (d c) -> p d c", d=N_CORES))
            nc.sync.dma_start(
                qsend_d[:, QA:QCH].rearrange("d (r c) -> r d c", r=DR),
                qpesend_sb.rearrange("r (d c) -> r d c", d=N_CORES))
            qrecv_d = dpool.tile([N_CORES, QCH], BF16, name="qrecv_d")
            nc.gpsimd.collective_compute("AllToAll", ALU.bypass, RG,
                                         [qsend_d[:, :]], [qrecv_d[:, :]])
            # land src-major (simple 3-dim DMA), then one DVE copy reorders so
            # the 16 head columns (src, hl) are contiguous per (j, bl) — the
            # scores lhsT slices must be plain 2-D APs for walrus
            qabs_raw = apool.tile([128, N_CORES * 4 * BPC * 2], BF16)
            nc.sync.dma_start(
                qabs_raw.rearrange("p (s c) -> p s c", s=N_CORES),
                qrecv_d[:, 0:QA].rearrange("s (p c) -> p s c", p=128))
            qpe_raw = apool.tile([DR, N_CORES * BPC * 2], BF16)
            nc.sync.dma_start(
                qpe_raw.rearrange("r (s c) -> r s c", s=N_CORES),
                qrecv_d[:, QA:QCH].rearrange("s (r c) -> r s c", r=DR))
            nc.vector.tensor_copy(
                qabsT.rearrange("p (j bl s hl) -> p s j bl hl",
                                j=4, bl=BPC, s=N_CORES),
                qabs_raw.rearrange("p (s j bl hl) -> p s j bl hl",
                                   s=N_CORES, j=4, bl=BPC))
            nc.vector.tensor_copy(
                qpeT_b16.rearrange("r (bl s hl) -> r s bl hl",
                                   bl=BPC, s=N_CORES),
                qpe_raw.rearrange("r (s bl hl) -> r s bl hl",
                                  s=N_CORES, bl=BPC))

            # latent for this core's own 4 sequences (W_kva replicated —
            # cheaper than a ReduceScatter gating the cache fixups)
            wkva_sb = apool.tile([128, 16 * (DC + DR)], BF16)
            nc.scalar.dma_start(wkva_sb.rearrange("p (i n) -> p i n", i=16),
                                t["wkva"].rearrange("(i p) n -> p i n", p=128))
            lat_ps = psA.tile([BPC, DC + DR], F32, tag="latps", bufs=1)
            for i in range(16):
                lhsT = hidkva_sb[:, i * BPC:(i + 1) * BPC]
                w0 = i * (DC + DR)
                nc.tensor.matmul(lat_ps[:, 0:DC], lhsT, wkva_sb[:, w0:w0 + DC],
                                 start=(i == 0), stop=(i == 15))
                nc.tensor.matmul(lat_ps[:, DC:DC + DR], lhsT,
                                 wkva_sb[:, w0 + DC:w0 + DC + DR],
                                 start=(i == 0), stop=(i == 15))
            lat_sb = apool.tile([BPC, DC + DR], F32)
            nc.scalar.copy(lat_sb, lat_ps)

            # rms_norm(latent[:, :512]) * ln_w
            sq = apool.tile([BPC, DC], F32)
            ssq = apool.tile([BPC, 1], F32)
            nc.scalar.activation(sq, lat_sb[:, :DC], AF.Square, accum_out=ssq)
            eps_sb = apool.tile([BPC, 1], F32)
            nc.vector.memset(eps_sb, EPS)
            stdv = apool.tile([BPC, 1], F32)
            nc.scalar.activation(stdv, ssq, AF.Sqrt, scale=1.0 / DC, bias=eps_sb)
            rinv = apool.tile([BPC, 1], F32)
            nc.vector.reciprocal(rinv, stdv)
            cn = apool.tile([BPC, DC], F32)
            nc.vector.tensor_scalar_mul(cn, lat_sb[:, :DC], rinv)
            nc.vector.tensor_mul(cn, cn, lnw_sb)
            nc.vector.tensor_copy(cn_b16, cn)
            for j in range(4):
                tp = psA.tile([128, BPC], F32, tag="small", bufs=2, name="tp")
                nc.tensor.transpose(tp, cn[:, j * 128:(j + 1) * 128],
                                    id_f32[0:BPC, 0:BPC])
                nc.vector.tensor_copy(cnT[:, j * BPC:(j + 1) * BPC], tp)

            # new-token k_pe: transpose then rope (cols)
            kpT = psA.tile([DR, BPC], F32, tag="small", bufs=2, name="kpT")
            nc.tensor.transpose(kpT, lat_sb[:, DC:DC + DR], id_f32[0:BPC, 0:BPC])
            kpe_f32 = apool.tile([DR, BPC], F32)
            nc.vector.tensor_copy(kpe_f32, kpT)
            rope_cols(kpe_f32, kpenT_b16, apool, "k")
        qa = qabsT.rearrange("p (j bl shl) -> p j bl shl", j=4, bl=BPC)
        qp = qpeT_b16.rearrange("r (bl shl) -> r bl shl", bl=BPC)

        wuv_sb = cpool.tile([128, NH * 4 * DV], BF16)
        nc.sync.dma_start(wuv_sb, t["wuv"][:, :])
        # W_O prefetch — emitted before the attention loop so the stream
        # overlaps the cache DMAs instead of sitting on the serial tail
        wo_tiles = []
        for h in range(NH):
            wo_t = wpool.tile([128, H], BF16, tag="wo", bufs=16, name="wo_t")
            nc.sync.dma_start(wo_t, t["wo"][h * DV:(h + 1) * DV, :])
            wo_tiles.append(wo_t)

        # ---------------- stage B: flash attention per sequence ----------------
        attn_sbs = []
        with tc.tile_pool(name="psB", bufs=1, space="PSUM") as psB, \
             tc.tile_pool(name="cachepool", bufs=2) as cachepool:
            for b in range(BPC):
                natv = t["ckv_nat"][b].rearrange("(g t p) c -> g t p c",
                                                 p=128, t=TQ)
                # ckv_t [512, 4096] viewed [p(c%128), j, k] for packed loads
                ckvTj = t["ckv_t"][b].rearrange("(j p) k -> p j k", p=128)
                kpeTv = t["kpe_t"][b]

                # kpe first (rope term of every quarter's scores needs it),
                # then per-quarter [ckvT, nat] pairs so the ring FIFO delivers
                # each quarter's scores operand before its attnV operand
                kt_ = cachepool.tile([DR, KVLEN], BF16, tag="kpeT", bufs=2,
                                     name="kt_")
                nc.scalar.dma_start(kt_, kpeTv[:, :])
                nc.vector.tensor_copy(kt_[:, KVLEN - 1:KVLEN],
                                      kpenT_b16[:, b:b + 1])

                probs = cachepool.tile([NH, KVLEN], BF16, tag="probs", bufs=2,
                                       name="probs")
                probsT = cachepool.tile([128, KT * NH], BF16, tag="probsT", bufs=2,
                                        name="probsT")
                den = wpool.tile([NH, NQ], F32, tag="den", bufs=2, name="den")
                attn_ps = psB.tile([NH, DC], F32, tag="attn", bufs=2, name="attn_ps")

                nats = []
                for q in range(NQ):
                    ct = cachepool.tile([128, 4 * KQ], BF16, tag="ckvT", bufs=3,
                                        name="ct")
                    ctv = ct.rearrange("p (j k) -> p j k", j=4)
                    nc.scalar.dma_start(ctv, ckvTj[:, :, q * KQ:(q + 1) * KQ])
                    nat = cachepool.tile([128, TQ * DC], BF16, tag="nat", bufs=3,
                                         name="nat")
                    nc.scalar.dma_start(nat.rearrange("p (t c) -> p t c", t=TQ),
                                        natv[q].rearrange("t p c -> p t c"))
                    nats.append(nat)
                    if q == NQ - 1:
                        for j in range(4):
                            nc.vector.tensor_copy(
                                ctv[:, j, KQ - 1:KQ],
                                cnT[:, j * BPC + b:j * BPC + b + 1])
                        # normed new-token latent into the last cache slot (row
                        # 127 of the last k-tile) — DMA for cross-partition move
                        nc.scalar.dma_start(nat[127:128, (TQ - 1) * DC:TQ * DC],
                                            cn_b16[b:b + 1, :])

                    sc = psB.tile([NH, KQ], F32, tag="scores", bufs=2, name="sc")
                    for half in range(2):
                        csl = slice(half * 512, (half + 1) * 512)
                        for j in range(4):
                            nc.tensor.matmul(sc[:, csl], qa[:, j, b, :],
                                             ctv[:, j, csl],
                                             start=(j == 0), stop=False)
                        nc.tensor.matmul(sc[:, csl], qp[:, b, :],
                                         kt_[:, q * KQ:(q + 1) * KQ][:, csl],
                                         start=False, stop=True)
                    # exp (softmax numerator) + running denominator
                    nc.scalar.activation(probs[:, q * KQ:(q + 1) * KQ], sc, AF.Exp,
                                         scale=SCALE, accum_out=den[:, q:q + 1])
                    pT = psB.tile([128, TQ * NH], BF16, tag="pT", bufs=2, name="pT")
                    for tl in range(TQ):
                        nc.tensor.transpose(
                            pT[:, tl * NH:(tl + 1) * NH],
                            probs[:, q * KQ + tl * 128:q * KQ + (tl + 1) * 128],
                            id_bf[0:NH, 0:NH])
                    nc.vector.tensor_copy(
                        probsT[:, q * TQ * NH:(q + 1) * TQ * NH], pT)
                    for tl in range(TQ):
                        tg = q * TQ + tl
                        nc.tensor.matmul(attn_ps,
                                         probsT[:, tg * NH:(tg + 1) * NH],
                                         nats[q][:, tl * DC:(tl + 1) * DC],
                                         start=(tg == 0), stop=(tg == KT - 1))

                dsum = wpool.tile([NH, 1], F32, tag="dsum", bufs=2, name="dsum")
                nc.vector.tensor_reduce(dsum, den, axis=mybir.AxisListType.X,
                                        op=ALU.add)
                rin = wpool.tile([NH, 1], F32, tag="rin", bufs=2, name="rin")
                nc.vector.reciprocal(rin, dsum)
                attn_sb = cpool.tile([NH, DC], F32, tag=f"attn{b}",
                                     name=f"attn_sb{b}")
                nc.scalar.activation(attn_sb, attn_ps, AF.Copy, scale=rin)
                attn_sbs.append(attn_sb)

        # ---------------- stage C: W_UV absorption + output projection ----------------
        with tc.tile_pool(name="psC", bufs=1, space="PSUM") as psC:
            attnT = cpool.tile([128, 4 * NH * BPC], BF16)   # [c%128, (j, h, b)]
            av = attnT.rearrange("p (j h b) -> p j h b", j=4, h=NH, b=BPC)
            for b in range(BPC):
                for j in range(4):
                    ap_ = psC.tile([128, NH], F32, tag="att", bufs=2, name="ap_")
                    nc.tensor.transpose(ap_, attn_sbs[b][:, j * 128:(j + 1) * 128],
                                        id_f32[0:NH, 0:NH])
                    nc.vector.tensor_copy(av[:, j, :, b], ap_)

            vT = cpool.tile([128, NH * BPC], BF16)          # [dv, (h, b)]
            wuv_v = wuv_sb.rearrange("p (h j v) -> p h j v", h=NH, j=4, v=DV)
            for h in range(NH):
                vps = psC.tile([128, BPC], F32, tag="vt", bufs=2, name="vps")
                for j in range(4):
                    nc.tensor.matmul(vps, wuv_v[:, h, j, :], av[:, j, h, :],
                                     start=(j == 0), stop=(j == 3))
                nc.vector.tensor_copy(vT[:, h * BPC:(h + 1) * BPC], vps)

            y_ps = [psC.tile([BPC, 512], F32, tag="y", bufs=4, name=f"y{n}")
                    for n in range(4)]
            for h in range(NH):
                for n in range(4):
                    nc.tensor.matmul(y_ps[n], vT[:, h * BPC:(h + 1) * BPC],
                                     wo_tiles[h][:, n * 512:(n + 1) * 512],
                                     start=(h == 0), stop=(h == NH - 1))
            y_sb = cpool.tile([BPC, H], F32)
            for n in range(4):
                nc.scalar.copy(y_sb[:, n * 512:(n + 1) * 512], y_ps[n])
            nc.sync.dma_start(t["out"][:, :], y_sb)


def build_module(debug=False):
    nc = bacc.Bacc("TRN2", target_bir_lowering=False, debug=debug,
                   num_devices=N_CORES)
    t = {}
    t["ckv_nat"] = nc.dram_tensor("ckv_nat", [BPC, KVLEN, DC], BF16,
                                  kind="ExternalInput")
    t["ckv_t"] = nc.dram_tensor("ckv_t", [BPC, DC, KVLEN], BF16,
                                kind="ExternalInput")
    t["kpe_t"] = nc.dram_tensor("kpe_t", [BPC, DR, KVLEN], BF16,
                                kind="ExternalInput")
    t["hidT"] = nc.dram_tensor("hidT", [128, 16 * BSZ], BF16,
                               kind="ExternalInput")
    t["hidT_kva"] = nc.dram_tensor("hidT_kva", [128, 16 * BPC], BF16,
                                   kind="ExternalInput")
    t["wuqr"] = nc.dram_tensor("wuqr", [H, 2 * DQ], BF16, kind="ExternalInput")
    t["wukt"] = nc.dram_tensor("wukt", [128, 2 * DC], BF16,
                               kind="ExternalInput")
    t["wkva"] = nc.dram_tensor("wkva", [H, DC + DR], BF16,
                               kind="ExternalInput")
    t["wuv"] = nc.dram_tensor("wuv", [128, NH * 4 * DV], BF16,
                              kind="ExternalInput")
    t["wo"] = nc.dram_tensor("wo", [NH * DV, H], BF16, kind="ExternalInput")
    t["lnw"] = nc.dram_tensor("lnw", [BPC, DC], F32, kind="ExternalInput")
    t["cosT"] = nc.dram_tensor("cosT", [DR, 1], F32, kind="ExternalInput")
    t["sinT"] = nc.dram_tensor("sinT", [DR, 1], F32, kind="ExternalInput")
    t["out"] = nc.dram_tensor("out", [BPC, H], F32, kind="ExternalOutput")

    with tile.TileContext(nc) as tc:
        _emit(tc, t)
    nc.compile()
    return nc


def prep_inputs(hidden_states, compressed_kv_normed_cache, k_pe_cache,
                W_UQR, W_kva, ln_w, W_UK, W_UV, W_O, cos, sin):
    """Host-side layout/dtype prep + per-core sharding. Returns in_maps."""
    bf16 = ml_dtypes.bfloat16
    f32 = np.float32

    # W_UK [h, c, d] -> [d, (h c)]
    wukt_full = np.ascontiguousarray(
        np.asarray(W_UK).transpose(2, 0, 1)).astype(bf16)       # [128, 16, 512]
    wuqr_h = np.asarray(W_UQR).reshape(H, NH, DQ)
    wkva_full = np.ascontiguousarray(np.asarray(W_kva)).astype(bf16)
    # W_UV [h, c, v] -> [c%128, (h, j, v)]
    wuv = np.asarray(W_UV).reshape(NH, 4, 128, DV).transpose(2, 0, 1, 3)
    wuv = np.ascontiguousarray(wuv.reshape(128, NH * 4 * DV)).astype(bf16)
    wo = np.ascontiguousarray(np.asarray(W_O)).astype(bf16)
    lnw = np.tile(np.asarray(ln_w, dtype=f32)[None, :], (BPC, 1))
    cosT = np.ascontiguousarray(np.asarray(cos, dtype=f32).reshape(1, DR).T)
    sinT = np.ascontiguousarray(np.asarray(sin, dtype=f32).reshape(1, DR).T)

    ckv = np.asarray(compressed_kv_normed_cache)
    kpe = np.asarray(k_pe_cache)
    hs = np.asarray(hidden_states)

    ckv_nat = ckv.astype(bf16)                                   # [32, k, c]
    ckv_t = ckv.transpose(0, 2, 1).astype(bf16)                  # [32, c, k]
    ckv_t = np.ascontiguousarray(ckv_t)
    kpe_t = np.ascontiguousarray(kpe.transpose(0, 2, 1).astype(bf16))

    # hiddenT for all 32 sequences: [128, (i 16, B 32)]
    hidT3 = hs.T.reshape(16, 128, BSZ)
    hidT_full = np.ascontiguousarray(
        hidT3.transpose(1, 0, 2).reshape(128, 16 * BSZ)).astype(bf16)

    in_maps = []
    for c in range(N_CORES):
        sl = slice(c * BPC, (c + 1) * BPC)
        hid_kva = np.ascontiguousarray(
            hs[sl].T.reshape(16, 128, BPC).transpose(1, 0, 2).reshape(
                128, 16 * BPC)).astype(bf16)
        wuqr_c = np.ascontiguousarray(
            wuqr_h[:, 2 * c:2 * c + 2, :].reshape(H, 2 * DQ)).astype(bf16)
        wukt_c = np.ascontiguousarray(
            wukt_full[:, 2 * c:2 * c + 2, :].reshape(128, 2 * DC))
        wkva_c = wkva_full
        in_maps.append({
            "ckv_nat": np.ascontiguousarray(ckv_nat[sl]),
            "ckv_t": np.ascontiguousarray(ckv_t[sl]),
            "kpe_t": np.ascontiguousarray(kpe_t[sl]),
            "hidT": hidT_full, "hidT_kva": hid_kva,
            "wuqr": wuqr_c, "wukt": wukt_c, "wkva": wkva_c, "wuv": wuv,
            "wo": wo,
            "lnw": lnw.astype(f32), "cosT": cosT.astype(f32),
            "sinT": sinT.astype(f32),
        })
    return in_maps


_MODULE = None


def _get_module():
    global _MODULE
    if _MODULE is None:
        _MODULE = build_module()
    return _MODULE


def kernel(**inputs):
    nc = _get_module()
    in_maps = prep_inputs(**inputs)
    res = run_bass_kernel_spmd(nc, in_maps, core_ids=list(range(N_CORES)))
    out = np.concatenate([r["out"] for r in res.results], axis=0)
    return np.ascontiguousarray(out.astype(np.float32))
